# revision 6
# baseline (speedup 1.0000x reference)
"""DeepGAT (4-layer GAT + BN + residual + MLP head) on 8 Trainium2 cores.

Sharding: nodes are dst-partitioned across the 8 cores (1250 nodes/core).
Edges are routed on the host to the core owning their dst node and sorted by
dst. Weights are replicated. Per layer each core projects all N nodes
(replicated compute), writes per-node rows [xl bf16 x1024 | alpha_src x8 |
pad] to its local DRAM, then gathers the rows of its edges' src nodes with
dma_gather. The per-chunk one-hot (edge, dst) matrices and their transposes
are precomputed on the host and stay SBUF-resident across layers; alpha_dst
is computed per dst block locally and broadcast to edges with a small
ohT @ ad matmul (no per-edge dst gather). Segment softmax normalization is
applied after aggregation (mathematically identical). h is exchanged with a
bf16 AllGather.
"""

import numpy as np
from jax import numpy as jnp

import concourse.bass as bass
import concourse.bacc as bacc
import concourse.mybir as mybir
from concourse.tile import TileContext
from concourse.tile_rust import add_dep_helper

FP32 = mybir.dt.float32
BF16 = mybir.dt.bfloat16
I16 = mybir.dt.int16
AF = mybir.ActivationFunctionType
OP = mybir.AluOpType

# problem constants (hardcoded per harness contract)
ALPHA = 0.1
BN_EPS = 1e-5
NEG_SLOPE = 0.2
HID = 128  # partition width; fixed

N, E, IN, H, L, CLS, M = 10000, 160000, 512, 8, 4, 2, 8
NPC = N // M                      # 1250 nodes per core
NPC_PAD = -(-NPC // 128) * 128    # 1280
NBLK = NPC_PAD // 128             # 10 local dst blocks
N_PAD = -(-N // 128) * 128        # 10112
NNB = N_PAD // 128                # 79 global node blocks
HC = H * HID                      # 1024
PRJ = HC + H                      # 1032: xl | alpha_src
ROW = -(-PRJ // 128) * 128        # 1152 (x2B = 2304B, %256 ok)
SZMAX = 8                         # chunks per gather split (128*8=1024 idxs)

BF = jnp.bfloat16


class Cfg:
    """Static schedule computed from the actual edge data."""

    def __init__(self, chunks_per_block):
        self.chunks_per_block = list(chunks_per_block)
        self.CH = sum(self.chunks_per_block)
        self.TOTE = 128 * self.CH


def _pack_idx16(idx, pad_to=None):
    """Pack int16 indices for dma_gather: idx i at [i%16, i//16], replicated
    to 128 partitions."""
    idx = np.asarray(idx, np.int64)
    n = len(idx)
    if pad_to is not None:
        assert pad_to >= n
        idx = np.concatenate([idx, np.zeros(pad_to - n, np.int64)])
        n = pad_to
    assert n % 16 == 0
    a = idx.astype(np.int16).reshape(n // 16, 16).T  # [16, n//16]
    return np.tile(a, (8, 1)).copy()  # [128, n//16]


def preprocess(x, edge_index, Wp, bp, Wl, att_src, att_dst, bl, gamma, beta,
               W1, b1, W2, b2):
    """Host-side: edge routing/sorting per core + one-hots + weight folding."""
    x = np.asarray(x, np.float32)
    src = np.concatenate([np.asarray(edge_index[0]), np.arange(N)]).astype(np.int64)
    dst = np.concatenate([np.asarray(edge_index[1]), np.arange(N)]).astype(np.int64)

    per_core = []
    for k in range(M):
        m = (dst // NPC) == k
        s_k, d_k = src[m], dst[m] - k * NPC
        order = np.argsort(d_k, kind="stable")
        per_core.append((s_k[order], d_k[order]))

    counts = np.zeros((M, NBLK), np.int64)
    for k in range(M):
        _, d_k = per_core[k]
        b = d_k // 128
        for bb in range(NBLK):
            counts[k, bb] = int((b == bb).sum())
    chunks_per_block = [max(1, int(np.ceil(counts[:, bb].max() / 128)))
                       for bb in range(NBLK)]
    cfg = Cfg(chunks_per_block)
    CH = cfg.CH

    per_core_inputs = []
    for k in range(M):
        s_k, d_k = per_core[k]
        b_k = d_k // 128
        srcidx = np.zeros(cfg.TOTE, np.int64)
        ohs = np.zeros((CH, 128, 128), np.float32)   # [chunk, edge, dstslot]
        off = 0  # in chunks
        for bb in range(NBLK):
            sel = b_k == bb
            cnt = int(sel.sum())
            cap = 128 * cfg.chunks_per_block[bb]
            assert cnt <= cap, (k, bb, cnt, cap)
            srcidx[off * 128:off * 128 + cnt] = s_k[sel]
            dloc = (d_k[sel] - 128 * bb).astype(np.int64)   # 0..127
            mm = np.arange(cnt)
            ohs[off + mm // 128, mm % 128, dloc] = 1.0
            off += cfg.chunks_per_block[bb]
        assert off == CH

        # SBUF layouts: oh [128 edge-part, CH, 128 dst], ohT [128 dst-part, CH, 128 edge]
        oh_sb = np.ascontiguousarray(ohs.transpose(1, 0, 2)).reshape(128, CH * 128)
        ohT_sb = np.ascontiguousarray(ohs.transpose(2, 0, 1)).reshape(128, CH * 128)

        xT_own = np.zeros((IN, NPC_PAD), np.float32)
        xT_own[:, :NPC] = x[k * NPC:(k + 1) * NPC].T

        per_core_inputs.append({
            "srcidx": _pack_idx16(srcidx),
            "oh_in": jnp.asarray(oh_sb, BF),
            "ohT_in": jnp.asarray(ohT_sb, BF),
            "xT_own": jnp.asarray(xT_own, BF),
        })

    # weight folding
    Wl = np.asarray(Wl, np.float32)          # [L, HID, HC]
    a_s = np.asarray(att_src, np.float32)    # [L, H, HID]
    a_d = np.asarray(att_dst, np.float32)
    Wcat = np.zeros((L, HID, PRJ), np.float32)
    Wad = np.zeros((L, HID, H), np.float32)
    for i in range(L):
        Wcat[i, :, :HC] = Wl[i]
        w3 = Wl[i].reshape(HID, H, HID)
        Wcat[i, :, HC:] = np.einsum("khc,hc->kh", w3, a_s[i])
        Wad[i] = np.einsum("khc,hc->kh", w3, a_d[i])

    bn_inv = 1.0 / np.sqrt(1.0 + BN_EPS)
    gamma = np.asarray(gamma, np.float32)
    beta = np.asarray(beta, np.float32)
    bl = np.asarray(bl, np.float32)
    # h = elu((1-a)*(gamma*bn_inv*(mean+bl)+beta) + a*prev); fold 1/H into s.
    s_aff = ((1.0 - ALPHA) * gamma * bn_inv / H).T.copy()            # [HID, L]
    t_aff = ((1.0 - ALPHA) * (gamma * bn_inv * bl + beta)).T.copy()  # [HID, L]

    ident = np.eye(128, dtype=np.float32)

    shared = {
        "Wp": jnp.asarray(np.asarray(Wp, np.float32), BF),
        "bp": np.asarray(bp, np.float32)[:, None],
        "Wcat": jnp.asarray(Wcat, BF),
        "Wad": jnp.asarray(Wad, BF),
        "s_aff": s_aff, "t_aff": t_aff,
        "W1": jnp.asarray(np.asarray(W1, np.float32), BF),
        "b1": np.asarray(b1, np.float32)[:, None],
        "W2": jnp.asarray(np.asarray(W2, np.float32), BF),
        "b2": np.asarray(b2, np.float32)[:, None],
        "ident": ident,
    }
    return cfg, shared, per_core_inputs


def _elu(nc, p, out_ap, z_ap, shape, tg):
    """out = elu(z) = relu(z) + exp(min(z,0)) - 1, z in SBUF f32."""
    P, F = shape
    mn = p.tile([P, F], FP32, tag=f"elu_mn_{tg}", name=f"elu_mn_{tg}")
    ex = p.tile([P, F], FP32, tag=f"elu_ex_{tg}", name=f"elu_ex_{tg}")
    rl = p.tile([P, F], FP32, tag=f"elu_rl_{tg}", name=f"elu_rl_{tg}")
    nc.vector.tensor_scalar_min(out=mn[:], in0=z_ap, scalar1=0.0)
    nc.scalar.activation(out=ex[:], in_=mn[:], func=AF.Exp)
    nc.vector.tensor_scalar_max(out=rl[:], in0=z_ap, scalar1=0.0)
    nc.vector.tensor_tensor(out=rl[:], in0=rl[:], in1=ex[:], op=OP.add)
    nc.vector.tensor_scalar_sub(out=out_ap, in0=rl[:], scalar1=1.0)


def build(nc, cfg):
    """Emit the SPMD program."""
    CH = cfg.CH
    qd = HID // 2

    # ---------------- I/O ----------------
    srcidx = nc.dram_tensor("srcidx", [128, cfg.TOTE // 16], I16, kind="ExternalInput")
    oh_in = nc.dram_tensor("oh_in", [128, CH * 128], BF16, kind="ExternalInput")
    ohT_in = nc.dram_tensor("ohT_in", [128, CH * 128], BF16, kind="ExternalInput")
    xT_own_in = nc.dram_tensor("xT_own", [IN, NPC_PAD], BF16, kind="ExternalInput")
    Wp_in = nc.dram_tensor("Wp", [IN, HID], BF16, kind="ExternalInput")
    bp_in = nc.dram_tensor("bp", [HID, 1], FP32, kind="ExternalInput")
    Wcat_in = nc.dram_tensor("Wcat", [L, HID, PRJ], BF16, kind="ExternalInput")
    Wad_in = nc.dram_tensor("Wad", [L, HID, H], BF16, kind="ExternalInput")
    s_aff_in = nc.dram_tensor("s_aff", [HID, L], FP32, kind="ExternalInput")
    t_aff_in = nc.dram_tensor("t_aff", [HID, L], FP32, kind="ExternalInput")
    W1_in = nc.dram_tensor("W1", [HID, qd], BF16, kind="ExternalInput")
    b1_in = nc.dram_tensor("b1", [qd, 1], FP32, kind="ExternalInput")
    W2_in = nc.dram_tensor("W2", [qd, CLS], BF16, kind="ExternalInput")
    b2_in = nc.dram_tensor("b2", [CLS, 1], FP32, kind="ExternalInput")
    ident_in = nc.dram_tensor("ident", [128, 128], FP32, kind="ExternalInput")
    out_dram = nc.dram_tensor("out", [CLS, NPC_PAD], FP32, kind="ExternalOutput")

    xlrow = nc.dram_tensor("xlrow", [N_PAD, ROW], BF16)
    bounce = nc.dram_tensor("h_bounce", [HID, NPC_PAD], BF16)
    agout = nc.dram_tensor("h_agout", [M * HID, NPC_PAD], BF16,
                           addr_space="Shared")

    with TileContext(nc) as tc:
        with (
            tc.tile_pool(name="const", bufs=1) as cpool,
            tc.tile_pool(name="hbuf", bufs=1) as hpool,
            tc.tile_pool(name="proj", bufs=2) as ppool,
            tc.tile_pool(name="gath", bufs=2) as gpool,
            tc.tile_pool(name="edge", bufs=2) as epool,
            tc.tile_pool(name="msg", bufs=3) as mpool,
            tc.tile_pool(name="blk", bufs=2) as bpool,
            tc.tile_pool(name="psP", bufs=2, space="PSUM") as psP,
            tc.tile_pool(name="psA", bufs=2, space="PSUM") as psA,
            tc.tile_pool(name="psD", bufs=2, space="PSUM") as psD,
        ):
            _regs = {}

            def nreg(v):
                if v not in _regs:
                    _regs[v] = nc.gpsimd.to_reg(v)
                return _regs[v]

            # ---------------- resident constants / state ----------------
            ident_f = cpool.tile([128, 128], FP32)
            nc.sync.dma_start(out=ident_f[:], in_=ident_in[:, :])
            srcidx_sb = cpool.tile([128, cfg.TOTE // 16], I16)
            nc.sync.dma_start(out=srcidx_sb[:], in_=srcidx[:, :])
            oh_sb = cpool.tile([128, CH, 128], BF16)
            nc.sync.dma_start(out=oh_sb[:].rearrange("p a b -> p (a b)"),
                              in_=oh_in[:, :])
            ohT_sb = cpool.tile([128, CH, 128], BF16)
            nc.sync.dma_start(out=ohT_sb[:].rearrange("p a b -> p (a b)"),
                              in_=ohT_in[:, :])
            s_aff = cpool.tile([128, L], FP32)
            nc.sync.dma_start(out=s_aff[:], in_=s_aff_in[:, :])
            t_aff = cpool.tile([128, L], FP32)
            nc.sync.dma_start(out=t_aff[:], in_=t_aff_in[:, :])
            W1_sb = cpool.tile([128, qd], BF16)
            nc.sync.dma_start(out=W1_sb[:], in_=W1_in[:, :])
            b1_sb = cpool.tile([qd, 1], FP32)
            nc.sync.dma_start(out=b1_sb[:], in_=b1_in[:, :])
            W2_sb = cpool.tile([qd, CLS], BF16)
            nc.sync.dma_start(out=W2_sb[:], in_=W2_in[:, :])
            b2_sb = cpool.tile([CLS, 1], FP32)
            nc.sync.dma_start(out=b2_sb[:], in_=b2_in[:, :])
            bp_sb = cpool.tile([HID, 1], FP32)
            nc.sync.dma_start(out=bp_sb[:], in_=bp_in[:, :])

            hT = hpool.tile([128, N_PAD], BF16, tag="hT")
            if N_PAD > N:
                nc.vector.memset(hT[:, N:], 0.0)
            h_bf = [hpool.tile([128, NPC_PAD], BF16, tag=f"h_bf{i}",
                               name=f"h_bf{i}")
                    for i in range(2)]

            kchunks = IN // 128

            # ------- h0 = elu(x @ Wp + bp), own nodes only (scoped pool) ----
            with tc.tile_pool(name="x0", bufs=2) as x0pool:
                Wp_sb = x0pool.tile([128, kchunks, HID], BF16, bufs=1)
                for kc in range(kchunks):
                    nc.sync.dma_start(out=Wp_sb[:, kc, :],
                                      in_=Wp_in[kc * 128:(kc + 1) * 128, :])
                for j0 in range(0, NPC_PAD, 512):
                    j1 = min(j0 + 512, NPC_PAD)
                    ps = psP.tile([128, j1 - j0], FP32, tag="p512", name="h0ps")
                    for kc in range(kchunks):
                        xt = x0pool.tile([128, 512], BF16, tag="xT", name="xT")
                        nc.sync.dma_start(
                            out=xt[:, :j1 - j0],
                            in_=xT_own_in[kc * 128:(kc + 1) * 128, j0:j1])
                        nc.tensor.matmul(out=ps[:], lhsT=Wp_sb[:, kc, :],
                                         rhs=xt[:, :j1 - j0],
                                         start=(kc == 0),
                                         stop=(kc == kchunks - 1))
                    z0 = x0pool.tile([128, 512], FP32, tag="z0", name="z0")
                    nc.scalar.activation(out=z0[:, :j1 - j0], in_=ps[:],
                                         func=AF.Identity,
                                         bias=bp_sb[:, :1], scale=1.0)
                    _elu(nc, x0pool, h_bf[0][:, j0:j1], z0[:, :j1 - j0],
                         (128, j1 - j0), "w")

            bw0 = nc.sync.dma_start(out=bounce[:, :], in_=h_bf[0][:])
            prev_bounce_writes = [bw0]
            prev_gathers = []
            prev_readbacks = []

            # ---------------- layers ----------------
            for li in range(L):
                hprev = h_bf[li % 2]
                hnew = h_bf[(li + 1) % 2]

                # --- alpha_dst for own nodes (uses hprev only; overlaps cc) ---
                Wad_t = ppool.tile([128, H], BF16, tag="Wad")
                nc.sync.dma_start(out=Wad_t[:], in_=Wad_in[li, :, :])
                adall = ppool.tile([128, NBLK, H], BF16, tag="adall")
                for bb in range(NBLK):
                    ps_ad = psD.tile([128, H], FP32, tag="small", name="ps_ad")
                    nc.tensor.matmul(out=ps_ad[:],
                                     lhsT=hprev[:, bb * 128:(bb + 1) * 128],
                                     rhs=Wad_t[:], start=True, stop=True)
                    nc.scalar.activation(out=adall[:, bb, :], in_=ps_ad[:],
                                         func=AF.Copy)

                # --- allgather h (own cols -> full hT) ---
                cc = nc.gpsimd.collective_compute(
                    "AllGather", OP.bypass,
                    replica_groups=[list(range(M))],
                    ins=[bounce[:, :]], outs=[agout[:, :]],
                )
                # order collective after everything that read/wrote the
                # exchanged buffers last layer (WAR/race fix)
                for w_ in prev_bounce_writes:
                    add_dep_helper(cc.ins, w_.ins, True, "bounce->cc")
                for g_ in prev_gathers:
                    add_dep_helper(cc.ins, g_.ins, True, "gather->cc")
                for r_ in prev_readbacks:
                    add_dep_helper(cc.ins, r_.ins, True, "readback->cc")
                readbacks = []
                for k in range(M):
                    d = nc.sync.dma_start(
                        out=hT[:, k * NPC:(k + 1) * NPC],
                        in_=agout[k * HID:(k + 1) * HID, :NPC])
                    add_dep_helper(d.ins, cc.ins, True, "cc->readback")
                    readbacks.append(d)
                prev_readbacks = readbacks

                # --- projection: all nodes, row = [xl | alpha_src | pad] ---
                Wc = ppool.tile([128, PRJ], BF16, tag="Wc")
                nc.sync.dma_start(out=Wc[:], in_=Wcat_in[li, :, :])
                tbl_writes = []
                for nb in range(NNB):
                    xlwr = ppool.tile([128, ROW], BF16, tag="xlwr")
                    ps1 = psP.tile([128, 512], FP32, tag="p512", name="ps1")
                    nc.tensor.matmul(out=ps1[:],
                                     lhsT=hT[:, nb * 128:(nb + 1) * 128],
                                     rhs=Wc[:, 0:512], start=True, stop=True)
                    nc.scalar.activation(out=xlwr[:, 0:512], in_=ps1[:],
                                         func=AF.Copy)
                    ps2 = psP.tile([128, 512], FP32, tag="p512", name="ps2")
                    nc.tensor.matmul(out=ps2[:],
                                     lhsT=hT[:, nb * 128:(nb + 1) * 128],
                                     rhs=Wc[:, 512:1024], start=True, stop=True)
                    nc.scalar.activation(out=xlwr[:, 512:1024], in_=ps2[:],
                                         func=AF.Copy)
                    ps3 = psD.tile([128, H], FP32, tag="small", name="ps3")
                    nc.tensor.matmul(out=ps3[:],
                                     lhsT=hT[:, nb * 128:(nb + 1) * 128],
                                     rhs=Wc[:, 1024:1032], start=True, stop=True)
                    nc.scalar.activation(out=xlwr[:, 1024:1032], in_=ps3[:],
                                         func=AF.Copy)
                    # cols 1032:1152 are never read downstream; left as-is
                    w_ = nc.sync.dma_start(
                        out=xlrow[nb * 128:(nb + 1) * 128, :], in_=xlwr[:])
                    tbl_writes.append(w_)

                # --- edge phase, per dst block ---
                gathers = []
                bounce_writes = []
                off = 0
                for bb in range(NBLK):
                    cb = cfg.chunks_per_block[bb]
                    splits = []
                    lo = 0
                    while lo < cb:
                        sz = min(SZMAX, cb - lo)
                        splits.append((lo, sz))
                        lo += sz

                    # alpha_dst broadcast to edges: svd[e, c*8+h]
                    svd_ps = psD.tile([128, cb * H], FP32, tag="small",
                                      name="svd_ps")
                    for c in range(cb):
                        nc.tensor.matmul(out=svd_ps[:, c * H:(c + 1) * H],
                                         lhsT=ohT_sb[:, off + c, :],
                                         rhs=adall[:, bb, :],
                                         start=True, stop=True)

                    den = psD.tile([128, H], FP32, tag="small", name="den")
                    agg = psA.tile([128, HC], FP32, tag="agg")

                    for (lo, sz) in splits:
                        g = gpool.tile([128, SZMAX, ROW], BF16, tag="gt",
                                       name="gt")
                        g1_ = nc.gpsimd.dma_gather(
                            out_ap=g[:, :sz, :], in_ap=xlrow[:, :],
                            idxs_ap=srcidx_sb[:, (off + lo) * 8:(off + lo + sz) * 8],
                            num_idxs=128 * sz, num_idxs_reg=nreg(128 * sz),
                            elem_size=ROW, single_packet=128 * sz <= 1024)
                        for w_ in tbl_writes:
                            add_dep_helper(g1_.ins, w_.ins, True, "tbl->gather")
                        gathers.append(g1_)

                        # u = alpha_src[src] + alpha_dst[dst]  (batched)
                        u = epool.tile([128, SZMAX, H], FP32, tag="u", name="u")
                        nc.vector.tensor_tensor(
                            out=u[:, :sz, :], in0=g[:, :sz, HC:HC + H],
                            in1=svd_ps[:, lo * H:(lo + sz) * H].rearrange(
                                "p (a b) -> p a b", a=sz),
                            op=OP.add)
                        # lrelu(u) = max(u, slope*u)
                        lr = epool.tile([128, SZMAX, H], FP32, tag="lr",
                                        name="lr")
                        nc.vector.tensor_scalar_mul(
                            out=lr[:, :sz, :], in0=u[:, :sz, :],
                            scalar1=NEG_SLOPE)
                        nc.vector.tensor_tensor(out=lr[:, :sz, :],
                                                in0=u[:, :sz, :],
                                                in1=lr[:, :sz, :], op=OP.max)
                        pe = epool.tile([128, SZMAX, H, 1], BF16, tag="pe",
                                        name="pe")
                        nc.scalar.activation(
                            out=pe[:, :sz].rearrange("p a b c -> p (a b c)"),
                            in_=lr[:, :sz].rearrange("p a b -> p (a b)"),
                            func=AF.Exp)

                        for j in range(sz):
                            c = lo + j
                            msg = mpool.tile([128, HC], BF16, tag="msg",
                                             name="msg")
                            nc.vector.tensor_tensor(
                                out=msg[:].rearrange("p (a b) -> p a b", a=H),
                                in0=g[:, j, :HC].rearrange(
                                    "p (a b) -> p a b", a=H),
                                in1=pe[:, j].to_broadcast([128, H, HID]),
                                op=OP.mult)
                            first, last = c == 0, c == cb - 1
                            nc.tensor.matmul(out=den[:],
                                             lhsT=oh_sb[:, off + c, :],
                                             rhs=pe[:, j, :, 0],
                                             start=first, stop=last,
                                             skip_group_check=True)
                            for j0 in range(0, HC, 512):
                                nc.tensor.matmul(out=agg[:, j0:j0 + 512],
                                                 lhsT=oh_sb[:, off + c, :],
                                                 rhs=msg[:, j0:j0 + 512],
                                                 start=first, stop=last,
                                                 skip_group_check=True)
                    off += cb

                    # --- block epilogue ---
                    rec = bpool.tile([128, H], FP32, tag="rec")
                    # pad dst lanes have denom 0; clamp (their agg is 0 too)
                    nc.vector.tensor_scalar_max(out=rec[:], in0=den[:],
                                                scalar1=1e-20)
                    nc.vector.reciprocal(out=rec[:], in_=rec[:])
                    wsum = bpool.tile([128, HC], FP32, tag="wsum")
                    for hh in range(H):
                        nc.vector.tensor_tensor(
                            out=wsum[:, hh * HID:(hh + 1) * HID],
                            in0=agg[:, hh * HID:(hh + 1) * HID],
                            in1=rec[:, hh:hh + 1].to_broadcast([128, HID]),
                            op=OP.mult)
                    nc.vector.tensor_tensor(out=wsum[:, 0:512],
                                            in0=wsum[:, 0:512],
                                            in1=wsum[:, 512:1024], op=OP.add)
                    nc.vector.tensor_tensor(out=wsum[:, 0:256],
                                            in0=wsum[:, 0:256],
                                            in1=wsum[:, 256:512], op=OP.add)
                    nc.vector.tensor_tensor(out=wsum[:, 0:128],
                                            in0=wsum[:, 0:128],
                                            in1=wsum[:, 128:256], op=OP.add)
                    mT_ps = psP.tile([128, 128], FP32, tag="p512", name="mT_ps")
                    nc.tensor.transpose(out=mT_ps[:], in_=wsum[:, 0:128],
                                        identity=ident_f[:])
                    z1 = bpool.tile([128, 128], FP32, tag="z1")
                    nc.scalar.activation(out=z1[:], in_=mT_ps[:],
                                         func=AF.Identity,
                                         bias=t_aff[:, li:li + 1],
                                         scale=s_aff[:, li:li + 1])
                    z2 = bpool.tile([128, 128], FP32, tag="z2")
                    nc.vector.tensor_scalar_mul(
                        out=z2[:], in0=hprev[:, bb * 128:(bb + 1) * 128],
                        scalar1=ALPHA)
                    nc.vector.tensor_tensor(out=z1[:], in0=z1[:], in1=z2[:],
                                            op=OP.add)
                    _elu(nc, bpool, hnew[:, bb * 128:(bb + 1) * 128], z1[:],
                         (128, 128), "n")
                    if li < L - 1:
                        bw = nc.sync.dma_start(
                            out=bounce[:, bb * 128:(bb + 1) * 128],
                            in_=hnew[:, bb * 128:(bb + 1) * 128])
                        add_dep_helper(bw.ins, cc.ins, True, "cc->bounce")
                        bounce_writes.append(bw)
                prev_gathers = gathers
                prev_bounce_writes = bounce_writes

            # ---------------- classifier ----------------
            hfin = h_bf[L % 2]
            with tc.tile_pool(name="cls", bufs=2) as wpool:
                osb = wpool.tile([CLS, NPC_PAD], FP32, tag="osb", bufs=1)
                for j0 in range(0, NPC_PAD, 512):
                    j1 = min(j0 + 512, NPC_PAD)
                    hid_ps = psP.tile([qd, j1 - j0], FP32, tag="p512",
                                      name="hid_ps")
                    nc.tensor.matmul(out=hid_ps[:], lhsT=W1_sb[:],
                                     rhs=hfin[:, j0:j1], start=True, stop=True)
                    zc = wpool.tile([qd, 512], FP32, tag="zc", name="zc")
                    nc.scalar.activation(out=zc[:, :j1 - j0], in_=hid_ps[:],
                                         func=AF.Identity,
                                         bias=b1_sb[:, :1], scale=1.0)
                    hidsb = wpool.tile([qd, 512], BF16, tag="hidsb",
                                       name="hidsb")
                    _elu(nc, wpool, hidsb[:, :j1 - j0], zc[:, :j1 - j0],
                         (qd, j1 - j0), "c")
                    out_ps = psP.tile([CLS, j1 - j0], FP32, tag="p512",
                                      name="out_ps")
                    nc.tensor.matmul(out=out_ps[:], lhsT=W2_sb[:],
                                     rhs=hidsb[:, :j1 - j0],
                                     start=True, stop=True)
                    nc.scalar.activation(out=osb[:, j0:j1], in_=out_ps[:],
                                         func=AF.Identity,
                                         bias=b2_sb[:, :1], scale=1.0)
                nc.sync.dma_start(out=out_dram[:, :], in_=osb[:])

    return nc


_LAST_EXEC_NS = None


def _run(inputs, trace=False):
    global _LAST_EXEC_NS
    from concourse.bass_utils import run_bass_kernel_spmd

    cfg, shared, per_core = preprocess(**inputs)
    nc = bacc.Bacc("TRN2", target_bir_lowering=False, debug=False,
                   num_devices=M)
    build(nc, cfg)
    nc.compile()

    in_maps = []
    for k in range(M):
        m = dict(shared)
        m.update(per_core[k])
        in_maps.append({k2: np.ascontiguousarray(v) for k2, v in m.items()})

    res = run_bass_kernel_spmd(nc, in_maps, list(range(M)), trace=trace)
    _LAST_EXEC_NS = res.exec_time_ns

    out = np.zeros((N, CLS), np.float32)
    for k in range(M):
        o = res.results[k]["out"]  # [CLS, NPC_PAD]
        out[k * NPC:(k + 1) * NPC] = o[:CLS, :NPC].T
    return out


def kernel(**inputs):
    return _run(inputs, trace=False)


# revision 10
# speedup vs baseline: 1.2711x; 1.2711x over previous
"""DeepGAT (4-layer GAT + BN + residual + MLP head) on 8 Trainium2 cores.

Sharding: nodes are dst-partitioned across the 8 cores (1250 nodes/core).
Edges are routed on the host to the core owning their dst node and sorted by
dst. Weights are replicated. Per layer each core projects all N nodes
(replicated compute), writes per-node rows [xl bf16 x1024 | alpha_src x8 |
pad] to its local DRAM, then gathers the rows of its edges' src nodes with
dma_gather. The per-chunk one-hot (edge, dst) matrices and their transposes
are precomputed on the host and stay SBUF-resident across layers; alpha_dst
is computed per dst block locally and broadcast to edges with a small
ohT @ ad matmul (no per-edge dst gather). Segment softmax normalization is
applied after aggregation (mathematically identical). h is exchanged with a
bf16 AllGather.
"""

import numpy as np
from jax import numpy as jnp

import concourse.bass as bass
import concourse.bacc as bacc
import concourse.mybir as mybir
from concourse.tile import TileContext
from concourse.tile_rust import add_dep_helper

FP32 = mybir.dt.float32
BF16 = mybir.dt.bfloat16
I16 = mybir.dt.int16
AF = mybir.ActivationFunctionType
OP = mybir.AluOpType

# problem constants (hardcoded per harness contract)
ALPHA = 0.1
BN_EPS = 1e-5
NEG_SLOPE = 0.2
HID = 128  # partition width; fixed

N, E, IN, H, L, CLS, M = 10000, 160000, 512, 8, 4, 2, 8
NPC = N // M                      # 1250 nodes per core
NPC_PAD = -(-NPC // 128) * 128    # 1280
NBLK = NPC_PAD // 128             # 10 local dst blocks
N_PAD = -(-N // 128) * 128        # 10112
NNB = N_PAD // 128                # 79 global node blocks
HC = H * HID                      # 1024
PRJ = HC + H                      # 1032: xl | alpha_src
ROW = -(-PRJ // 128) * 128        # 1152 (x2B = 2304B, %256 ok)
SZMAX = 8                         # chunks per gather split (128*8=1024 idxs)

BF = jnp.bfloat16


class Cfg:
    """Static schedule computed from the actual edge data."""

    def __init__(self, chunks_per_block):
        self.chunks_per_block = list(chunks_per_block)
        self.CH = sum(self.chunks_per_block)
        self.TOTE = 128 * self.CH


def _pack_idx16(idx, pad_to=None):
    """Pack int16 indices for dma_gather: idx i at [i%16, i//16], replicated
    to 128 partitions."""
    idx = np.asarray(idx, np.int64)
    n = len(idx)
    if pad_to is not None:
        assert pad_to >= n
        idx = np.concatenate([idx, np.zeros(pad_to - n, np.int64)])
        n = pad_to
    assert n % 16 == 0
    a = idx.astype(np.int16).reshape(n // 16, 16).T  # [16, n//16]
    return np.tile(a, (8, 1)).copy()  # [128, n//16]


def preprocess(x, edge_index, Wp, bp, Wl, att_src, att_dst, bl, gamma, beta,
               W1, b1, W2, b2):
    """Host-side: edge routing/sorting per core + one-hots + weight folding."""
    x = np.asarray(x, np.float32)
    src = np.concatenate([np.asarray(edge_index[0]), np.arange(N)]).astype(np.int64)
    dst = np.concatenate([np.asarray(edge_index[1]), np.arange(N)]).astype(np.int64)

    per_core = []
    for k in range(M):
        m = (dst // NPC) == k
        s_k, d_k = src[m], dst[m] - k * NPC
        order = np.argsort(d_k, kind="stable")
        per_core.append((s_k[order], d_k[order]))

    counts = np.zeros((M, NBLK), np.int64)
    for k in range(M):
        _, d_k = per_core[k]
        b = d_k // 128
        for bb in range(NBLK):
            counts[k, bb] = int((b == bb).sum())
    chunks_per_block = [max(1, int(np.ceil(counts[:, bb].max() / 128)))
                       for bb in range(NBLK)]
    cfg = Cfg(chunks_per_block)
    CH = cfg.CH

    per_core_inputs = []
    for k in range(M):
        s_k, d_k = per_core[k]
        b_k = d_k // 128
        srcidx = np.zeros(cfg.TOTE, np.int64)
        ohs = np.zeros((CH, 128, 128), np.float32)   # [chunk, edge, dstslot]
        off = 0  # in chunks
        for bb in range(NBLK):
            sel = b_k == bb
            cnt = int(sel.sum())
            cap = 128 * cfg.chunks_per_block[bb]
            assert cnt <= cap, (k, bb, cnt, cap)
            srcidx[off * 128:off * 128 + cnt] = s_k[sel]
            dloc = (d_k[sel] - 128 * bb).astype(np.int64)   # 0..127
            mm = np.arange(cnt)
            ohs[off + mm // 128, mm % 128, dloc] = 1.0
            off += cfg.chunks_per_block[bb]
        assert off == CH

        # SBUF layouts: oh [128 edge-part, CH, 128 dst], ohT [128 dst-part, CH, 128 edge]
        oh_sb = np.ascontiguousarray(ohs.transpose(1, 0, 2)).reshape(128, CH * 128)
        ohT_sb = np.ascontiguousarray(ohs.transpose(2, 0, 1)).reshape(128, CH * 128)

        xT_own = np.zeros((IN, NPC_PAD), np.float32)
        xT_own[:, :NPC] = x[k * NPC:(k + 1) * NPC].T

        per_core_inputs.append({
            "srcidx": _pack_idx16(srcidx),
            "oh_in": jnp.asarray(oh_sb, BF),
            "ohT_in": jnp.asarray(ohT_sb, BF),
            "xT_own": jnp.asarray(xT_own, BF),
        })

    # weight folding
    Wl = np.asarray(Wl, np.float32)          # [L, HID, HC]
    a_s = np.asarray(att_src, np.float32)    # [L, H, HID]
    a_d = np.asarray(att_dst, np.float32)
    Wcat = np.zeros((L, HID, PRJ), np.float32)
    Wad = np.zeros((L, HID, H), np.float32)
    for i in range(L):
        # channel-major interleave: col c*H+h holds head h, channel c, so the
        # per-edge attention broadcast has a packed (stride-1) last dim on DVE
        Wcat[i, :, :HC] = Wl[i].reshape(HID, H, HID).transpose(0, 2, 1).reshape(HID, HC)
        w3 = Wl[i].reshape(HID, H, HID)
        Wcat[i, :, HC:] = np.einsum("khc,hc->kh", w3, a_s[i])
        Wad[i] = np.einsum("khc,hc->kh", w3, a_d[i])

    bn_inv = 1.0 / np.sqrt(1.0 + BN_EPS)
    gamma = np.asarray(gamma, np.float32)
    beta = np.asarray(beta, np.float32)
    bl = np.asarray(bl, np.float32)
    # h = elu((1-a)*(gamma*bn_inv*(mean+bl)+beta) + a*prev); fold 1/H into s.
    s_aff = ((1.0 - ALPHA) * gamma * bn_inv / H).T.copy()            # [HID, L]
    t_aff = ((1.0 - ALPHA) * (gamma * bn_inv * bl + beta)).T.copy()  # [HID, L]

    ident = np.eye(128, dtype=np.float32)

    shared = {
        "Wp": jnp.asarray(np.asarray(Wp, np.float32), BF),
        "bp": np.asarray(bp, np.float32)[:, None],
        "Wcat": jnp.asarray(Wcat, BF),
        "Wad": jnp.asarray(Wad, BF),
        "s_aff": s_aff, "t_aff": t_aff,
        "W1": jnp.asarray(np.asarray(W1, np.float32), BF),
        "b1": np.asarray(b1, np.float32)[:, None],
        "W2": jnp.asarray(np.asarray(W2, np.float32), BF),
        "b2": np.asarray(b2, np.float32)[:, None],
        "ident": ident,
    }
    return cfg, shared, per_core_inputs


def _elu(nc, p, out_ap, z_ap, shape, tg):
    """out = elu(z) = relu(z) + exp(min(z,0)) - 1, z in SBUF f32."""
    P, F = shape
    mn = p.tile([P, F], FP32, tag=f"elu_mn_{tg}", name=f"elu_mn_{tg}")
    ex = p.tile([P, F], FP32, tag=f"elu_ex_{tg}", name=f"elu_ex_{tg}")
    rl = p.tile([P, F], FP32, tag=f"elu_rl_{tg}", name=f"elu_rl_{tg}")
    nc.vector.tensor_scalar_min(out=mn[:], in0=z_ap, scalar1=0.0)
    nc.scalar.activation(out=ex[:], in_=mn[:], func=AF.Exp)
    nc.vector.tensor_scalar_max(out=rl[:], in0=z_ap, scalar1=0.0)
    nc.vector.tensor_tensor(out=rl[:], in0=rl[:], in1=ex[:], op=OP.add)
    nc.vector.tensor_scalar_sub(out=out_ap, in0=rl[:], scalar1=1.0)


def build(nc, cfg):
    """Emit the SPMD program."""
    CH = cfg.CH
    qd = HID // 2

    # ---------------- I/O ----------------
    srcidx = nc.dram_tensor("srcidx", [128, cfg.TOTE // 16], I16, kind="ExternalInput")
    oh_in = nc.dram_tensor("oh_in", [128, CH * 128], BF16, kind="ExternalInput")
    ohT_in = nc.dram_tensor("ohT_in", [128, CH * 128], BF16, kind="ExternalInput")
    xT_own_in = nc.dram_tensor("xT_own", [IN, NPC_PAD], BF16, kind="ExternalInput")
    Wp_in = nc.dram_tensor("Wp", [IN, HID], BF16, kind="ExternalInput")
    bp_in = nc.dram_tensor("bp", [HID, 1], FP32, kind="ExternalInput")
    Wcat_in = nc.dram_tensor("Wcat", [L, HID, PRJ], BF16, kind="ExternalInput")
    Wad_in = nc.dram_tensor("Wad", [L, HID, H], BF16, kind="ExternalInput")
    s_aff_in = nc.dram_tensor("s_aff", [HID, L], FP32, kind="ExternalInput")
    t_aff_in = nc.dram_tensor("t_aff", [HID, L], FP32, kind="ExternalInput")
    W1_in = nc.dram_tensor("W1", [HID, qd], BF16, kind="ExternalInput")
    b1_in = nc.dram_tensor("b1", [qd, 1], FP32, kind="ExternalInput")
    W2_in = nc.dram_tensor("W2", [qd, CLS], BF16, kind="ExternalInput")
    b2_in = nc.dram_tensor("b2", [CLS, 1], FP32, kind="ExternalInput")
    ident_in = nc.dram_tensor("ident", [128, 128], FP32, kind="ExternalInput")
    out_dram = nc.dram_tensor("out", [CLS, NPC_PAD], FP32, kind="ExternalOutput")

    xlrow = nc.dram_tensor("xlrow", [N_PAD, ROW], BF16)
    bounce = nc.dram_tensor("h_bounce", [HID, NPC_PAD], BF16)
    agout = nc.dram_tensor("h_agout", [M * HID, NPC_PAD], BF16,
                           addr_space="Shared")

    with TileContext(nc) as tc:
        with (
            tc.tile_pool(name="const", bufs=1) as cpool,
            tc.tile_pool(name="hbuf", bufs=1) as hpool,
            tc.tile_pool(name="proj", bufs=2) as ppool,
            tc.tile_pool(name="gath", bufs=2) as gpool,
            tc.tile_pool(name="edge", bufs=2) as epool,
            tc.tile_pool(name="msg", bufs=3) as mpool,
            tc.tile_pool(name="blk", bufs=2) as bpool,
            tc.tile_pool(name="psP", bufs=2, space="PSUM") as psP,
            tc.tile_pool(name="psA", bufs=2, space="PSUM") as psA,
            tc.tile_pool(name="psD", bufs=2, space="PSUM") as psD,
        ):
            _regs = {}

            def nreg(v):
                if v not in _regs:
                    _regs[v] = nc.gpsimd.to_reg(v)
                return _regs[v]

            # ---------------- resident constants / state ----------------
            ident_f = cpool.tile([128, 128], FP32)
            nc.sync.dma_start(out=ident_f[:], in_=ident_in[:, :])
            srcidx_sb = cpool.tile([128, cfg.TOTE // 16], I16)
            nc.sync.dma_start(out=srcidx_sb[:], in_=srcidx[:, :])
            oh_sb = cpool.tile([128, CH, 128], BF16)
            nc.sync.dma_start(out=oh_sb[:].rearrange("p a b -> p (a b)"),
                              in_=oh_in[:, :])
            ohT_sb = cpool.tile([128, CH, 128], BF16)
            nc.sync.dma_start(out=ohT_sb[:].rearrange("p a b -> p (a b)"),
                              in_=ohT_in[:, :])
            s_aff = cpool.tile([128, L], FP32)
            nc.sync.dma_start(out=s_aff[:], in_=s_aff_in[:, :])
            t_aff = cpool.tile([128, L], FP32)
            nc.sync.dma_start(out=t_aff[:], in_=t_aff_in[:, :])
            W1_sb = cpool.tile([128, qd], BF16)
            nc.sync.dma_start(out=W1_sb[:], in_=W1_in[:, :])
            b1_sb = cpool.tile([qd, 1], FP32)
            nc.sync.dma_start(out=b1_sb[:], in_=b1_in[:, :])
            W2_sb = cpool.tile([qd, CLS], BF16)
            nc.sync.dma_start(out=W2_sb[:], in_=W2_in[:, :])
            b2_sb = cpool.tile([CLS, 1], FP32)
            nc.sync.dma_start(out=b2_sb[:], in_=b2_in[:, :])
            bp_sb = cpool.tile([HID, 1], FP32)
            nc.sync.dma_start(out=bp_sb[:], in_=bp_in[:, :])

            hT = hpool.tile([128, N_PAD], BF16, tag="hT")
            if N_PAD > N:
                nc.vector.memset(hT[:, N:], 0.0)
            h_bf = [hpool.tile([128, NPC_PAD], BF16, tag=f"h_bf{i}",
                               name=f"h_bf{i}")
                    for i in range(2)]

            kchunks = IN // 128

            # ------- h0 = elu(x @ Wp + bp), own nodes only (scoped pool) ----
            with tc.tile_pool(name="x0", bufs=2) as x0pool:
                Wp_sb = x0pool.tile([128, kchunks, HID], BF16, bufs=1)
                for kc in range(kchunks):
                    nc.sync.dma_start(out=Wp_sb[:, kc, :],
                                      in_=Wp_in[kc * 128:(kc + 1) * 128, :])
                for j0 in range(0, NPC_PAD, 512):
                    j1 = min(j0 + 512, NPC_PAD)
                    ps = psP.tile([128, j1 - j0], FP32, tag="p512", name="h0ps")
                    for kc in range(kchunks):
                        xt = x0pool.tile([128, 512], BF16, tag="xT", name="xT")
                        nc.sync.dma_start(
                            out=xt[:, :j1 - j0],
                            in_=xT_own_in[kc * 128:(kc + 1) * 128, j0:j1])
                        nc.tensor.matmul(out=ps[:], lhsT=Wp_sb[:, kc, :],
                                         rhs=xt[:, :j1 - j0],
                                         start=(kc == 0),
                                         stop=(kc == kchunks - 1))
                    z0 = x0pool.tile([128, 512], FP32, tag="z0", name="z0")
                    nc.scalar.activation(out=z0[:, :j1 - j0], in_=ps[:],
                                         func=AF.Identity,
                                         bias=bp_sb[:, :1], scale=1.0)
                    _elu(nc, x0pool, h_bf[0][:, j0:j1], z0[:, :j1 - j0],
                         (128, j1 - j0), "w")

            bw0 = nc.sync.dma_start(out=bounce[:, :], in_=h_bf[0][:])
            prev_bounce_writes = [bw0]
            prev_gathers = []
            prev_readbacks = []

            # ---------------- layers ----------------
            for li in range(L):
                hprev = h_bf[li % 2]
                hnew = h_bf[(li + 1) % 2]

                # --- alpha_dst for own nodes (uses hprev only; overlaps cc) ---
                Wad_t = ppool.tile([128, H], BF16, tag="Wad")
                nc.sync.dma_start(out=Wad_t[:], in_=Wad_in[li, :, :])
                adall = ppool.tile([128, NBLK, H], BF16, tag="adall")
                for bb in range(NBLK):
                    ps_ad = psD.tile([128, H], FP32, tag="small", name="ps_ad")
                    nc.tensor.matmul(out=ps_ad[:],
                                     lhsT=hprev[:, bb * 128:(bb + 1) * 128],
                                     rhs=Wad_t[:], start=True, stop=True)
                    nc.scalar.activation(out=adall[:, bb, :], in_=ps_ad[:],
                                         func=AF.Copy)

                # --- allgather h (own cols -> full hT) ---
                cc = nc.gpsimd.collective_compute(
                    "AllGather", OP.bypass,
                    replica_groups=[list(range(M))],
                    ins=[bounce[:, :]], outs=[agout[:, :]],
                )
                # order collective after everything that read/wrote the
                # exchanged buffers last layer (WAR/race fix)
                for w_ in prev_bounce_writes:
                    add_dep_helper(cc.ins, w_.ins, True, "bounce->cc")
                for g_ in prev_gathers:
                    add_dep_helper(cc.ins, g_.ins, True, "gather->cc")
                for r_ in prev_readbacks:
                    add_dep_helper(cc.ins, r_.ins, True, "readback->cc")
                readbacks = []
                for k in range(M):
                    d = nc.sync.dma_start(
                        out=hT[:, k * NPC:(k + 1) * NPC],
                        in_=agout[k * HID:(k + 1) * HID, :NPC])
                    add_dep_helper(d.ins, cc.ins, True, "cc->readback")
                    readbacks.append(d)
                prev_readbacks = readbacks

                # --- projection: all nodes, row = [xl | alpha_src | pad] ---
                Wc = ppool.tile([128, PRJ], BF16, tag="Wc")
                nc.sync.dma_start(out=Wc[:], in_=Wcat_in[li, :, :])
                tbl_writes = []
                for nb in range(NNB):
                    xlwr = ppool.tile([128, ROW], BF16, tag="xlwr")
                    ps1 = psP.tile([128, 512], FP32, tag="p512", name="ps1")
                    nc.tensor.matmul(out=ps1[:],
                                     lhsT=hT[:, nb * 128:(nb + 1) * 128],
                                     rhs=Wc[:, 0:512], start=True, stop=True)
                    nc.vector.tensor_copy(out=xlwr[:, 0:512], in_=ps1[:])
                    ps2 = psP.tile([128, 512], FP32, tag="p512", name="ps2")
                    nc.tensor.matmul(out=ps2[:],
                                     lhsT=hT[:, nb * 128:(nb + 1) * 128],
                                     rhs=Wc[:, 512:1024], start=True, stop=True)
                    nc.scalar.activation(out=xlwr[:, 512:1024], in_=ps2[:],
                                         func=AF.Copy)
                    ps3 = psD.tile([128, H], FP32, tag="small", name="ps3")
                    nc.tensor.matmul(out=ps3[:],
                                     lhsT=hT[:, nb * 128:(nb + 1) * 128],
                                     rhs=Wc[:, 1024:1032], start=True, stop=True)
                    nc.scalar.activation(out=xlwr[:, 1024:1032], in_=ps3[:],
                                         func=AF.Copy)
                    # cols 1032:1152 are never read downstream; left as-is
                    w_ = nc.sync.dma_start(
                        out=xlrow[nb * 128:(nb + 1) * 128, :], in_=xlwr[:])
                    tbl_writes.append(w_)

                # --- edge phase, per dst block ---
                gathers = []
                bounce_writes = []
                off = 0
                for bb in range(NBLK):
                    cb = cfg.chunks_per_block[bb]
                    splits = []
                    lo = 0
                    while lo < cb:
                        sz = min(SZMAX, cb - lo)
                        splits.append((lo, sz))
                        lo += sz

                    # alpha_dst broadcast to edges: svd[e, c*8+h]
                    svd_ps = psD.tile([128, cb * H], FP32, tag="small",
                                      name="svd_ps")
                    for c in range(cb):
                        nc.tensor.matmul(out=svd_ps[:, c * H:(c + 1) * H],
                                         lhsT=ohT_sb[:, off + c, :],
                                         rhs=adall[:, bb, :],
                                         start=True, stop=True)

                    den = psD.tile([128, H], FP32, tag="small", name="den")
                    agg = psA.tile([128, HC], FP32, tag="agg")

                    for (lo, sz) in splits:
                        g = gpool.tile([128, SZMAX, ROW], BF16, tag="gt",
                                       name="gt")
                        g1_ = nc.gpsimd.dma_gather(
                            out_ap=g[:, :sz, :], in_ap=xlrow[:, :],
                            idxs_ap=srcidx_sb[:, (off + lo) * 8:(off + lo + sz) * 8],
                            num_idxs=128 * sz, num_idxs_reg=nreg(128 * sz),
                            elem_size=ROW, single_packet=128 * sz <= 1024)
                        for w_ in tbl_writes:
                            add_dep_helper(g1_.ins, w_.ins, True, "tbl->gather")
                        gathers.append(g1_)

                        # u = alpha_src[src] + alpha_dst[dst]  (batched)
                        u = epool.tile([128, SZMAX, H], FP32, tag="u", name="u")
                        nc.vector.tensor_tensor(
                            out=u[:, :sz, :], in0=g[:, :sz, HC:HC + H],
                            in1=svd_ps[:, lo * H:(lo + sz) * H].rearrange(
                                "p (a b) -> p a b", a=sz),
                            op=OP.add)
                        # lrelu(u) = max(u, slope*u)
                        lr = epool.tile([128, SZMAX, H], FP32, tag="lr",
                                        name="lr")
                        nc.vector.tensor_scalar_mul(
                            out=lr[:, :sz, :], in0=u[:, :sz, :],
                            scalar1=NEG_SLOPE)
                        nc.vector.tensor_tensor(out=lr[:, :sz, :],
                                                in0=u[:, :sz, :],
                                                in1=lr[:, :sz, :], op=OP.max)
                        pe = epool.tile([128, SZMAX, 1, H], BF16, tag="pe",
                                        name="pe")
                        nc.scalar.activation(
                            out=pe[:, :sz].rearrange("p a b c -> p (a b c)"),
                            in_=lr[:, :sz].rearrange("p a b -> p (a b)"),
                            func=AF.Exp)

                        for j in range(sz):
                            c = lo + j
                            # channel-major layout: packed stride-1 head dim
                            # on every operand -> DVE 2x_1p mode
                            msg = mpool.tile([128, HC], BF16, tag="msg",
                                             name="msg")
                            nc.vector.tensor_tensor(
                                out=msg[:].rearrange("p (a b) -> p a b", a=HID),
                                in0=g[:, j, :HC].rearrange(
                                    "p (a b) -> p a b", a=HID),
                                in1=pe[:, j].to_broadcast([128, HID, H]),
                                op=OP.mult)
                            first, last = c == 0, c == cb - 1
                            nc.tensor.matmul(out=den[:],
                                             lhsT=oh_sb[:, off + c, :],
                                             rhs=pe[:, j, 0, :],
                                             start=first, stop=last,
                                             skip_group_check=True)
                            for j0 in range(0, HC, 512):
                                nc.tensor.matmul(out=agg[:, j0:j0 + 512],
                                                 lhsT=oh_sb[:, off + c, :],
                                                 rhs=msg[:, j0:j0 + 512],
                                                 start=first, stop=last,
                                                 skip_group_check=True)
                    off += cb

                    # --- block epilogue (channel-major agg: [d, c*H+h]) ---
                    rec = bpool.tile([128, 1, H], FP32, tag="rec")
                    # pad dst lanes have denom 0; clamp (their agg is 0 too)
                    nc.vector.tensor_scalar_max(out=rec[:, 0, :], in0=den[:],
                                                scalar1=1e-20)
                    nc.vector.reciprocal(out=rec[:, 0, :], in_=rec[:, 0, :])
                    wsum = bpool.tile([128, HID, H], FP32, tag="wsum")
                    nc.vector.tensor_tensor(
                        out=wsum[:],
                        in0=agg[:].rearrange("p (a b) -> p a b", a=HID),
                        in1=rec[:].to_broadcast([128, HID, H]), op=OP.mult)
                    nc.vector.tensor_tensor(out=wsum[:, :, 0:4],
                                            in0=wsum[:, :, 0:4],
                                            in1=wsum[:, :, 4:8], op=OP.add)
                    nc.vector.tensor_tensor(out=wsum[:, :, 0:2],
                                            in0=wsum[:, :, 0:2],
                                            in1=wsum[:, :, 2:4], op=OP.add)
                    hm = bpool.tile([128, 128], FP32, tag="hm")
                    nc.vector.tensor_tensor(out=hm[:], in0=wsum[:, :, 0],
                                            in1=wsum[:, :, 1], op=OP.add)
                    mT_ps = psP.tile([128, 128], FP32, tag="p512", name="mT_ps")
                    nc.tensor.transpose(out=mT_ps[:], in_=hm[:],
                                        identity=ident_f[:])
                    z1 = bpool.tile([128, 128], FP32, tag="z1")
                    nc.scalar.activation(out=z1[:], in_=mT_ps[:],
                                         func=AF.Identity,
                                         bias=t_aff[:, li:li + 1],
                                         scale=s_aff[:, li:li + 1])
                    z2 = bpool.tile([128, 128], FP32, tag="z2")
                    nc.vector.tensor_scalar_mul(
                        out=z2[:], in0=hprev[:, bb * 128:(bb + 1) * 128],
                        scalar1=ALPHA)
                    nc.vector.tensor_tensor(out=z1[:], in0=z1[:], in1=z2[:],
                                            op=OP.add)
                    _elu(nc, bpool, hnew[:, bb * 128:(bb + 1) * 128], z1[:],
                         (128, 128), "n")
                    if li < L - 1:
                        bw = nc.sync.dma_start(
                            out=bounce[:, bb * 128:(bb + 1) * 128],
                            in_=hnew[:, bb * 128:(bb + 1) * 128])
                        add_dep_helper(bw.ins, cc.ins, True, "cc->bounce")
                        bounce_writes.append(bw)
                prev_gathers = gathers
                prev_bounce_writes = bounce_writes

            # ---------------- classifier ----------------
            hfin = h_bf[L % 2]
            with tc.tile_pool(name="cls", bufs=2) as wpool:
                osb = wpool.tile([CLS, NPC_PAD], FP32, tag="osb", bufs=1)
                for j0 in range(0, NPC_PAD, 512):
                    j1 = min(j0 + 512, NPC_PAD)
                    hid_ps = psP.tile([qd, j1 - j0], FP32, tag="p512",
                                      name="hid_ps")
                    nc.tensor.matmul(out=hid_ps[:], lhsT=W1_sb[:],
                                     rhs=hfin[:, j0:j1], start=True, stop=True)
                    zc = wpool.tile([qd, 512], FP32, tag="zc", name="zc")
                    nc.scalar.activation(out=zc[:, :j1 - j0], in_=hid_ps[:],
                                         func=AF.Identity,
                                         bias=b1_sb[:, :1], scale=1.0)
                    hidsb = wpool.tile([qd, 512], BF16, tag="hidsb",
                                       name="hidsb")
                    _elu(nc, wpool, hidsb[:, :j1 - j0], zc[:, :j1 - j0],
                         (qd, j1 - j0), "c")
                    out_ps = psP.tile([CLS, j1 - j0], FP32, tag="p512",
                                      name="out_ps")
                    nc.tensor.matmul(out=out_ps[:], lhsT=W2_sb[:],
                                     rhs=hidsb[:, :j1 - j0],
                                     start=True, stop=True)
                    nc.scalar.activation(out=osb[:, j0:j1], in_=out_ps[:],
                                         func=AF.Identity,
                                         bias=b2_sb[:, :1], scale=1.0)
                nc.sync.dma_start(out=out_dram[:, :], in_=osb[:])

    return nc


_LAST_EXEC_NS = None


def _run(inputs, trace=False):
    global _LAST_EXEC_NS
    from concourse.bass_utils import run_bass_kernel_spmd

    cfg, shared, per_core = preprocess(**inputs)
    nc = bacc.Bacc("TRN2", target_bir_lowering=False, debug=False,
                   num_devices=M)
    build(nc, cfg)
    nc.compile()

    in_maps = []
    for k in range(M):
        m = dict(shared)
        m.update(per_core[k])
        in_maps.append({k2: np.ascontiguousarray(v) for k2, v in m.items()})

    res = run_bass_kernel_spmd(nc, in_maps, list(range(M)), trace=trace)
    _LAST_EXEC_NS = res.exec_time_ns

    out = np.zeros((N, CLS), np.float32)
    for k in range(M):
        o = res.results[k]["out"]  # [CLS, NPC_PAD]
        out[k * NPC:(k + 1) * NPC] = o[:CLS, :NPC].T
    return out


def kernel(**inputs):
    return _run(inputs, trace=False)


# revision 18
# speedup vs baseline: 1.5828x; 1.2453x over previous
"""DeepGAT (4-layer GAT + BN + residual + MLP head) on 8 Trainium2 cores.

Sharding: nodes are dst-partitioned across the 8 cores (1250 nodes/core).
Edges are routed on the host to the core owning their dst node and sorted by
dst. Weights are replicated. Per layer each core projects all N nodes
(replicated compute), writes per-node rows [xl bf16 x1024 | alpha_src x8 |
pad] to its local DRAM, then gathers the rows of its edges' src nodes with
dma_gather. The per-chunk one-hot (edge, dst) matrices and their transposes
are precomputed on the host and stay SBUF-resident across layers; alpha_dst
is computed per dst block locally and broadcast to edges with a small
ohT @ ad matmul (no per-edge dst gather). Segment softmax normalization is
applied after aggregation (mathematically identical). h is exchanged with a
bf16 AllGather.
"""

import numpy as np
from jax import numpy as jnp

import concourse.bass as bass
import concourse.bacc as bacc
import concourse.mybir as mybir
from concourse.tile import TileContext
from concourse.tile_rust import add_dep_helper

FP32 = mybir.dt.float32
BF16 = mybir.dt.bfloat16
I16 = mybir.dt.int16
AF = mybir.ActivationFunctionType
OP = mybir.AluOpType

# problem constants (hardcoded per harness contract)
ALPHA = 0.1
BN_EPS = 1e-5
NEG_SLOPE = 0.2
HID = 128  # partition width; fixed

N, E, IN, H, L, CLS, M = 10000, 160000, 512, 8, 4, 2, 8
NPC = N // M                      # 1250 nodes per core
NPC_PAD = -(-NPC // 128) * 128    # 1280
NBLK = NPC_PAD // 128             # 10 local dst blocks
N_PAD = -(-N // 128) * 128        # 10112
NNB = N_PAD // 128                # 79 global node blocks
HC = H * HID                      # 1024
PRJ = HC + H                      # 1032: xl | alpha_src
ROW = -(-PRJ // 128) * 128        # 1152 (x2B = 2304B, %256 ok)
SZMAX = 8                         # chunks per gather split (128*8=1024 idxs)

BF = jnp.bfloat16


class Cfg:
    """Static schedule computed from the actual edge data."""

    def __init__(self, chunks_per_block):
        self.chunks_per_block = list(chunks_per_block)
        self.CH = sum(self.chunks_per_block)
        self.TOTE = 128 * self.CH


def _pack_idx16(idx, pad_to=None):
    """Pack int16 indices for dma_gather: idx i at [i%16, i//16], replicated
    to 128 partitions."""
    idx = np.asarray(idx, np.int64)
    n = len(idx)
    if pad_to is not None:
        assert pad_to >= n
        idx = np.concatenate([idx, np.zeros(pad_to - n, np.int64)])
        n = pad_to
    assert n % 16 == 0
    a = idx.astype(np.int16).reshape(n // 16, 16).T  # [16, n//16]
    return np.tile(a, (8, 1)).copy()  # [128, n//16]


def preprocess(x, edge_index, Wp, bp, Wl, att_src, att_dst, bl, gamma, beta,
               W1, b1, W2, b2):
    """Host-side: edge routing/sorting per core + one-hots + weight folding."""
    x = np.asarray(x, np.float32)
    src = np.concatenate([np.asarray(edge_index[0]), np.arange(N)]).astype(np.int64)
    dst = np.concatenate([np.asarray(edge_index[1]), np.arange(N)]).astype(np.int64)

    per_core = []
    for k in range(M):
        m = (dst // NPC) == k
        s_k, d_k = src[m], dst[m] - k * NPC
        order = np.argsort(d_k, kind="stable")
        per_core.append((s_k[order], d_k[order]))

    counts = np.zeros((M, NBLK), np.int64)
    for k in range(M):
        _, d_k = per_core[k]
        b = d_k // 128
        for bb in range(NBLK):
            counts[k, bb] = int((b == bb).sum())
    chunks_per_block = [max(1, int(np.ceil(counts[:, bb].max() / 128)))
                       for bb in range(NBLK)]
    cfg = Cfg(chunks_per_block)
    CH = cfg.CH

    per_core_inputs = []
    for k in range(M):
        s_k, d_k = per_core[k]
        b_k = d_k // 128
        srcidx = np.zeros(cfg.TOTE, np.int64)
        ohs = np.zeros((CH, 128, 128), np.float32)   # [chunk, edge, dstslot]
        off = 0  # in chunks
        for bb in range(NBLK):
            sel = b_k == bb
            cnt = int(sel.sum())
            cap = 128 * cfg.chunks_per_block[bb]
            assert cnt <= cap, (k, bb, cnt, cap)
            srcidx[off * 128:off * 128 + cnt] = s_k[sel]
            dloc = (d_k[sel] - 128 * bb).astype(np.int64)   # 0..127
            mm = np.arange(cnt)
            ohs[off + mm // 128, mm % 128, dloc] = 1.0
            off += cfg.chunks_per_block[bb]
        assert off == CH

        # SBUF layouts: oh [128 edge-part, CH, 128 dst], ohT [128 dst-part, CH, 128 edge]
        oh_sb = np.ascontiguousarray(ohs.transpose(1, 0, 2)).reshape(128, CH * 128)
        ohT_sb = np.ascontiguousarray(ohs.transpose(2, 0, 1)).reshape(128, CH * 128)

        xT_own = np.zeros((IN, NPC_PAD), np.float32)
        xT_own[:, :NPC] = x[k * NPC:(k + 1) * NPC].T

        per_core_inputs.append({
            "srcidx": _pack_idx16(srcidx),
            "oh_in": jnp.asarray(oh_sb, BF),
            "ohT_in": jnp.asarray(ohT_sb, BF),
            "xT_own": jnp.asarray(xT_own, BF),
        })

    # weight folding
    Wl = np.asarray(Wl, np.float32)          # [L, HID, HC]
    a_s = np.asarray(att_src, np.float32)    # [L, H, HID]
    a_d = np.asarray(att_dst, np.float32)
    Wcat = np.zeros((L, HID, PRJ), np.float32)
    Wad = np.zeros((L, HID, H), np.float32)
    for i in range(L):
        # channel-major interleave: col c*H+h holds head h, channel c, so the
        # per-edge attention broadcast has a packed (stride-1) last dim on DVE
        Wcat[i, :, :HC] = Wl[i].reshape(HID, H, HID).transpose(0, 2, 1).reshape(HID, HC)
        w3 = Wl[i].reshape(HID, H, HID)
        Wcat[i, :, HC:] = np.einsum("khc,hc->kh", w3, a_s[i])
        Wad[i] = np.einsum("khc,hc->kh", w3, a_d[i])

    bn_inv = 1.0 / np.sqrt(1.0 + BN_EPS)
    gamma = np.asarray(gamma, np.float32)
    beta = np.asarray(beta, np.float32)
    bl = np.asarray(bl, np.float32)
    # h = elu((1-a)*(gamma*bn_inv*(mean+bl)+beta) + a*prev); fold 1/H into s.
    s_aff = ((1.0 - ALPHA) * gamma * bn_inv / H).T.copy()            # [HID, L]
    t_aff = ((1.0 - ALPHA) * (gamma * bn_inv * bl + beta)).T.copy()  # [HID, L]

    ident = np.eye(128, dtype=np.float32)

    shared = {
        "Wp": jnp.asarray(np.asarray(Wp, np.float32), BF),
        "bp": np.asarray(bp, np.float32)[:, None],
        "Wcat": jnp.asarray(Wcat, BF),
        "Wad": jnp.asarray(Wad, BF),
        "s_aff": s_aff, "t_aff": t_aff,
        "W1": jnp.asarray(np.asarray(W1, np.float32), BF),
        "b1": np.asarray(b1, np.float32)[:, None],
        "W2": jnp.asarray(np.asarray(W2, np.float32), BF),
        "b2": np.asarray(b2, np.float32)[:, None],
        "ident": ident,
    }
    return cfg, shared, per_core_inputs


def _elu(nc, p, out_ap, z_ap, shape, tg):
    """out = elu(z) = relu(z) + exp(min(z,0)) - 1, z in SBUF f32."""
    P, F = shape
    mn = p.tile([P, F], FP32, tag=f"elu_mn_{tg}", name=f"elu_mn_{tg}")
    ex = p.tile([P, F], FP32, tag=f"elu_ex_{tg}", name=f"elu_ex_{tg}")
    rl = p.tile([P, F], FP32, tag=f"elu_rl_{tg}", name=f"elu_rl_{tg}")
    nc.vector.tensor_scalar_min(out=mn[:], in0=z_ap, scalar1=0.0)
    nc.scalar.activation(out=ex[:], in_=mn[:], func=AF.Exp)
    nc.vector.tensor_scalar_max(out=rl[:], in0=z_ap, scalar1=0.0)
    nc.vector.tensor_tensor(out=rl[:], in0=rl[:], in1=ex[:], op=OP.add)
    nc.vector.tensor_scalar_sub(out=out_ap, in0=rl[:], scalar1=1.0)


def build(nc, cfg):
    """Emit the SPMD program."""
    CH = cfg.CH
    qd = HID // 2

    # ---------------- I/O ----------------
    srcidx = nc.dram_tensor("srcidx", [128, cfg.TOTE // 16], I16, kind="ExternalInput")
    oh_in = nc.dram_tensor("oh_in", [128, CH * 128], BF16, kind="ExternalInput")
    ohT_in = nc.dram_tensor("ohT_in", [128, CH * 128], BF16, kind="ExternalInput")
    xT_own_in = nc.dram_tensor("xT_own", [IN, NPC_PAD], BF16, kind="ExternalInput")
    Wp_in = nc.dram_tensor("Wp", [IN, HID], BF16, kind="ExternalInput")
    bp_in = nc.dram_tensor("bp", [HID, 1], FP32, kind="ExternalInput")
    Wcat_in = nc.dram_tensor("Wcat", [L, HID, PRJ], BF16, kind="ExternalInput")
    Wad_in = nc.dram_tensor("Wad", [L, HID, H], BF16, kind="ExternalInput")
    s_aff_in = nc.dram_tensor("s_aff", [HID, L], FP32, kind="ExternalInput")
    t_aff_in = nc.dram_tensor("t_aff", [HID, L], FP32, kind="ExternalInput")
    W1_in = nc.dram_tensor("W1", [HID, qd], BF16, kind="ExternalInput")
    b1_in = nc.dram_tensor("b1", [qd, 1], FP32, kind="ExternalInput")
    W2_in = nc.dram_tensor("W2", [qd, CLS], BF16, kind="ExternalInput")
    b2_in = nc.dram_tensor("b2", [CLS, 1], FP32, kind="ExternalInput")
    ident_in = nc.dram_tensor("ident", [128, 128], FP32, kind="ExternalInput")
    out_dram = nc.dram_tensor("out", [CLS, NPC_PAD], FP32, kind="ExternalOutput")

    xlrow = nc.dram_tensor("xlrow", [N_PAD, ROW], BF16)
    bounce = nc.dram_tensor("h_bounce", [HID, NPC_PAD], BF16)
    agout = nc.dram_tensor("h_agout", [M * HID, NPC_PAD], BF16,
                           addr_space="Shared")

    with TileContext(nc) as tc:
        with (
            tc.tile_pool(name="const", bufs=1) as cpool,
            tc.tile_pool(name="hbuf", bufs=1) as hpool,
            tc.tile_pool(name="proj", bufs=2) as ppool,
            tc.tile_pool(name="gath", bufs=4) as gpool,
            tc.tile_pool(name="ohp", bufs=2) as ohpool,
            tc.tile_pool(name="edge", bufs=2) as epool,
            tc.tile_pool(name="msg", bufs=3) as mpool,
            tc.tile_pool(name="blk", bufs=2) as bpool,
            tc.tile_pool(name="psP", bufs=2, space="PSUM") as psP,
            tc.tile_pool(name="psA", bufs=2, space="PSUM") as psA,
            tc.tile_pool(name="psD", bufs=2, space="PSUM") as psD,
        ):
            _regs = {}

            def nreg(v):
                if v not in _regs:
                    _regs[v] = nc.gpsimd.to_reg(v)
                return _regs[v]

            # ---------------- resident constants / state ----------------
            ident_f = cpool.tile([128, 128], FP32)
            nc.sync.dma_start(out=ident_f[:], in_=ident_in[:, :])
            srcidx_sb = cpool.tile([128, cfg.TOTE // 16], I16)
            nc.sync.dma_start(out=srcidx_sb[:], in_=srcidx[:, :])
            s_aff = cpool.tile([128, L], FP32)
            nc.sync.dma_start(out=s_aff[:], in_=s_aff_in[:, :])
            t_aff = cpool.tile([128, L], FP32)
            nc.sync.dma_start(out=t_aff[:], in_=t_aff_in[:, :])
            W1_sb = cpool.tile([128, qd], BF16)
            nc.sync.dma_start(out=W1_sb[:], in_=W1_in[:, :])
            b1_sb = cpool.tile([qd, 1], FP32)
            nc.sync.dma_start(out=b1_sb[:], in_=b1_in[:, :])
            W2_sb = cpool.tile([qd, CLS], BF16)
            nc.sync.dma_start(out=W2_sb[:], in_=W2_in[:, :])
            b2_sb = cpool.tile([CLS, 1], FP32)
            nc.sync.dma_start(out=b2_sb[:], in_=b2_in[:, :])
            bp_sb = cpool.tile([HID, 1], FP32)
            nc.sync.dma_start(out=bp_sb[:], in_=bp_in[:, :])
            Wc_all = cpool.tile([128, L, PRJ], BF16)
            for li in range(L):
                nc.sync.dma_start(out=Wc_all[:, li, :], in_=Wcat_in[li, :, :])
            Wad_all = cpool.tile([128, L, H], BF16)
            for li in range(L):
                nc.sync.dma_start(out=Wad_all[:, li, :], in_=Wad_in[li, :, :])

            hT = hpool.tile([128, N_PAD], BF16, tag="hT")
            if N_PAD > N:
                nc.vector.memset(hT[:, N:], 0.0)
            h_bf = [hpool.tile([128, NPC_PAD], BF16, tag=f"h_bf{i}",
                               name=f"h_bf{i}")
                    for i in range(2)]

            kchunks = IN // 128

            # ------- h0 = elu(x @ Wp + bp), own nodes only (scoped pool) ----
            with tc.tile_pool(name="x0", bufs=2) as x0pool:
                Wp_sb = x0pool.tile([128, kchunks, HID], BF16, bufs=1)
                for kc in range(kchunks):
                    nc.sync.dma_start(out=Wp_sb[:, kc, :],
                                      in_=Wp_in[kc * 128:(kc + 1) * 128, :])
                for j0 in range(0, NPC_PAD, 512):
                    j1 = min(j0 + 512, NPC_PAD)
                    ps = psP.tile([128, j1 - j0], FP32, tag="p512", name="h0ps")
                    for kc in range(kchunks):
                        xt = x0pool.tile([128, 512], BF16, tag="xT", name="xT")
                        nc.sync.dma_start(
                            out=xt[:, :j1 - j0],
                            in_=xT_own_in[kc * 128:(kc + 1) * 128, j0:j1])
                        nc.tensor.matmul(out=ps[:], lhsT=Wp_sb[:, kc, :],
                                         rhs=xt[:, :j1 - j0],
                                         start=(kc == 0),
                                         stop=(kc == kchunks - 1))
                    z0 = x0pool.tile([128, 512], FP32, tag="z0", name="z0")
                    nc.scalar.activation(out=z0[:, :j1 - j0], in_=ps[:],
                                         func=AF.Identity,
                                         bias=bp_sb[:, :1], scale=1.0)
                    _elu(nc, x0pool, h_bf[0][:, j0:j1], z0[:, :j1 - j0],
                         (128, j1 - j0), "w")

            bw0 = nc.sync.dma_start(out=bounce[:, :], in_=h_bf[0][:])
            prev_bounce_writes = [bw0]
            prev_gathers = []
            prev_readbacks = []

            # ---------------- layers ----------------
            for li in range(L):
                hprev = h_bf[li % 2]
                hnew = h_bf[(li + 1) % 2]

                # --- alpha_dst for own nodes (uses hprev only; overlaps cc) ---
                adall = ppool.tile([128, NBLK, H], BF16, tag="adall")
                for bb in range(NBLK):
                    ps_ad = psD.tile([128, H], FP32, tag="small", name="ps_ad")
                    nc.tensor.matmul(out=ps_ad[:],
                                     lhsT=hprev[:, bb * 128:(bb + 1) * 128],
                                     rhs=Wad_all[:, li, :], start=True, stop=True)
                    nc.scalar.activation(out=adall[:, bb, :], in_=ps_ad[:],
                                         func=AF.Copy)

                # --- allgather h (own cols -> full hT) ---
                cc = nc.gpsimd.collective_compute(
                    "AllGather", OP.bypass,
                    replica_groups=[list(range(M))],
                    ins=[bounce[:, :]], outs=[agout[:, :]],
                )
                # order collective after everything that read/wrote the
                # exchanged buffers last layer (WAR/race fix)
                for w_ in prev_bounce_writes:
                    add_dep_helper(cc.ins, w_.ins, True, "bounce->cc")
                for g_ in prev_gathers:
                    add_dep_helper(cc.ins, g_.ins, True, "gather->cc")
                for r_ in prev_readbacks:
                    add_dep_helper(cc.ins, r_.ins, True, "readback->cc")
                readbacks = []
                for k in range(M):
                    d = nc.sync.dma_start(
                        out=hT[:, k * NPC:(k + 1) * NPC],
                        in_=agout[k * HID:(k + 1) * HID, :NPC])
                    add_dep_helper(d.ins, cc.ins, True, "cc->readback")
                    readbacks.append(d)
                prev_readbacks = readbacks

                # --- projection: all nodes, row = [xl | alpha_src | pad] ---
                Wc = Wc_all[:, li, :]
                tbl_writes = []
                for nb in range(NNB):
                    xlwr = ppool.tile([128, ROW], BF16, tag="xlwr", bufs=4)
                    ps1 = psP.tile([128, 512], FP32, tag="p512", name="ps1")
                    nc.tensor.matmul(out=ps1[:],
                                     lhsT=hT[:, nb * 128:(nb + 1) * 128],
                                     rhs=Wc[:, 0:512], start=True, stop=True)
                    nc.vector.tensor_copy(out=xlwr[:, 0:512], in_=ps1[:])
                    ps2 = psP.tile([128, 512], FP32, tag="p512", name="ps2")
                    nc.tensor.matmul(out=ps2[:],
                                     lhsT=hT[:, nb * 128:(nb + 1) * 128],
                                     rhs=Wc[:, 512:1024], start=True, stop=True)
                    nc.scalar.activation(out=xlwr[:, 512:1024], in_=ps2[:],
                                         func=AF.Copy)
                    ps3 = psD.tile([128, H], FP32, tag="small", name="ps3")
                    nc.tensor.matmul(out=ps3[:],
                                     lhsT=hT[:, nb * 128:(nb + 1) * 128],
                                     rhs=Wc[:, 1024:1032], start=True, stop=True)
                    nc.scalar.activation(out=xlwr[:, 1024:1032], in_=ps3[:],
                                         func=AF.Copy)
                    # cols 1032:1152 are never read downstream; left as-is
                    w_ = nc.sync.dma_start(
                        out=xlrow[nb * 128:(nb + 1) * 128, :], in_=xlwr[:])
                    tbl_writes.append(w_)

                # --- edge phase, per dst block ---
                gathers = []
                bounce_writes = []
                off = 0
                cbmax = max(cfg.chunks_per_block)
                for bb in range(NBLK):
                    cb = cfg.chunks_per_block[bb]
                    splits = []
                    lo = 0
                    while lo < cb:
                        sz = min(SZMAX, cb - lo)
                        splits.append((lo, sz))
                        lo += sz

                    oh_t = ohpool.tile([128, cbmax, 128], BF16, tag="oh",
                                       name="oh_t")
                    nc.sync.dma_start(
                        out=oh_t[:, :cb, :].rearrange("p a b -> p (a b)"),
                        in_=oh_in[:, off * 128:(off + cb) * 128])
                    ohT_t = ohpool.tile([128, cbmax, 128], BF16, tag="ohT",
                                        name="ohT_t")
                    nc.sync.dma_start(
                        out=ohT_t[:, :cb, :].rearrange("p a b -> p (a b)"),
                        in_=ohT_in[:, off * 128:(off + cb) * 128])

                    # alpha_dst broadcast to edges: svd[e, c*8+h]
                    svd_ps = psD.tile([128, cb * H], FP32, tag="small",
                                      name="svd_ps")
                    for c in range(cb):
                        nc.tensor.matmul(out=svd_ps[:, c * H:(c + 1) * H],
                                         lhsT=ohT_t[:, c, :],
                                         rhs=adall[:, bb, :],
                                         start=True, stop=True)

                    den = psD.tile([128, H], FP32, tag="small", name="den")
                    agg = psA.tile([128, HC], FP32, tag="agg")

                    for (lo, sz) in splits:
                        g = gpool.tile([128, SZMAX, ROW], BF16, tag="gt",
                                       name="gt")
                        g1_ = nc.gpsimd.dma_gather(
                            out_ap=g[:, :sz, :], in_ap=xlrow[:, :],
                            idxs_ap=srcidx_sb[:, (off + lo) * 8:(off + lo + sz) * 8],
                            num_idxs=128 * sz, num_idxs_reg=nreg(128 * sz),
                            elem_size=ROW, single_packet=128 * sz <= 1024)
                        for w_ in tbl_writes:
                            add_dep_helper(g1_.ins, w_.ins, True, "tbl->gather")
                        gathers.append(g1_)

                        # u = alpha_src[src] + alpha_dst[dst]  (batched)
                        u = epool.tile([128, SZMAX, H], FP32, tag="u", name="u")
                        nc.vector.tensor_tensor(
                            out=u[:, :sz, :], in0=g[:, :sz, HC:HC + H],
                            in1=svd_ps[:, lo * H:(lo + sz) * H].rearrange(
                                "p (a b) -> p a b", a=sz),
                            op=OP.add)
                        # lrelu(u) = max(u, slope*u)
                        lr = epool.tile([128, SZMAX, H], FP32, tag="lr",
                                        name="lr")
                        nc.vector.tensor_scalar_mul(
                            out=lr[:, :sz, :], in0=u[:, :sz, :],
                            scalar1=NEG_SLOPE)
                        nc.vector.tensor_tensor(out=lr[:, :sz, :],
                                                in0=u[:, :sz, :],
                                                in1=lr[:, :sz, :], op=OP.max)
                        pe = epool.tile([128, SZMAX, 1, H], BF16, tag="pe",
                                        name="pe")
                        nc.scalar.activation(
                            out=pe[:, :sz].rearrange("p a b c -> p (a b c)"),
                            in_=lr[:, :sz].rearrange("p a b -> p (a b)"),
                            func=AF.Exp)

                        for j in range(sz):
                            c = lo + j
                            # channel-major layout: packed stride-1 head dim
                            # on every operand -> DVE 2x_1p mode
                            msg = mpool.tile([128, HC], BF16, tag="msg",
                                             name="msg")
                            nc.vector.tensor_tensor(
                                out=msg[:].rearrange("p (a b) -> p a b", a=HID),
                                in0=g[:, j, :HC].rearrange(
                                    "p (a b) -> p a b", a=HID),
                                in1=pe[:, j].to_broadcast([128, HID, H]),
                                op=OP.mult)
                            first, last = c == 0, c == cb - 1
                            nc.tensor.matmul(out=den[:],
                                             lhsT=oh_t[:, c, :],
                                             rhs=pe[:, j, 0, :],
                                             start=first, stop=last,
                                             skip_group_check=True)
                            for j0 in range(0, HC, 512):
                                nc.tensor.matmul(out=agg[:, j0:j0 + 512],
                                                 lhsT=oh_t[:, c, :],
                                                 rhs=msg[:, j0:j0 + 512],
                                                 start=first, stop=last,
                                                 skip_group_check=True)
                    off += cb

                    # --- block epilogue (channel-major agg: [d, c*H+h]) ---
                    rec = bpool.tile([128, 1, H], FP32, tag="rec")
                    # pad dst lanes have denom 0; clamp (their agg is 0 too)
                    nc.vector.tensor_scalar_max(out=rec[:, 0, :], in0=den[:],
                                                scalar1=1e-20)
                    nc.vector.reciprocal(out=rec[:, 0, :], in_=rec[:, 0, :])
                    wsum = bpool.tile([128, HID, H], FP32, tag="wsum")
                    nc.vector.tensor_tensor(
                        out=wsum[:],
                        in0=agg[:].rearrange("p (a b) -> p a b", a=HID),
                        in1=rec[:].to_broadcast([128, HID, H]), op=OP.mult)
                    nc.vector.tensor_tensor(out=wsum[:, :, 0:4],
                                            in0=wsum[:, :, 0:4],
                                            in1=wsum[:, :, 4:8], op=OP.add)
                    nc.vector.tensor_tensor(out=wsum[:, :, 0:2],
                                            in0=wsum[:, :, 0:2],
                                            in1=wsum[:, :, 2:4], op=OP.add)
                    hm = bpool.tile([128, 128], FP32, tag="hm")
                    nc.vector.tensor_tensor(out=hm[:], in0=wsum[:, :, 0],
                                            in1=wsum[:, :, 1], op=OP.add)
                    mT_ps = psP.tile([128, 128], FP32, tag="p512", name="mT_ps")
                    nc.tensor.transpose(out=mT_ps[:], in_=hm[:],
                                        identity=ident_f[:])
                    z1 = bpool.tile([128, 128], FP32, tag="z1")
                    nc.scalar.activation(out=z1[:], in_=mT_ps[:],
                                         func=AF.Identity,
                                         bias=t_aff[:, li:li + 1],
                                         scale=s_aff[:, li:li + 1])
                    z2 = bpool.tile([128, 128], FP32, tag="z2")
                    nc.vector.tensor_scalar_mul(
                        out=z2[:], in0=hprev[:, bb * 128:(bb + 1) * 128],
                        scalar1=ALPHA)
                    nc.vector.tensor_tensor(out=z1[:], in0=z1[:], in1=z2[:],
                                            op=OP.add)
                    _elu(nc, bpool, hnew[:, bb * 128:(bb + 1) * 128], z1[:],
                         (128, 128), "n")
                    if li < L - 1:
                        bw = nc.sync.dma_start(
                            out=bounce[:, bb * 128:(bb + 1) * 128],
                            in_=hnew[:, bb * 128:(bb + 1) * 128])
                        add_dep_helper(bw.ins, cc.ins, True, "cc->bounce")
                        bounce_writes.append(bw)
                prev_gathers = gathers
                prev_bounce_writes = bounce_writes

            # ---------------- classifier ----------------
            hfin = h_bf[L % 2]
            with tc.tile_pool(name="cls", bufs=2) as wpool:
                osb = wpool.tile([CLS, NPC_PAD], FP32, tag="osb", bufs=1)
                for j0 in range(0, NPC_PAD, 512):
                    j1 = min(j0 + 512, NPC_PAD)
                    hid_ps = psP.tile([qd, j1 - j0], FP32, tag="p512",
                                      name="hid_ps")
                    nc.tensor.matmul(out=hid_ps[:], lhsT=W1_sb[:],
                                     rhs=hfin[:, j0:j1], start=True, stop=True)
                    zc = wpool.tile([qd, 512], FP32, tag="zc", name="zc")
                    nc.scalar.activation(out=zc[:, :j1 - j0], in_=hid_ps[:],
                                         func=AF.Identity,
                                         bias=b1_sb[:, :1], scale=1.0)
                    hidsb = wpool.tile([qd, 512], BF16, tag="hidsb",
                                       name="hidsb")
                    _elu(nc, wpool, hidsb[:, :j1 - j0], zc[:, :j1 - j0],
                         (qd, j1 - j0), "c")
                    out_ps = psP.tile([CLS, j1 - j0], FP32, tag="p512",
                                      name="out_ps")
                    nc.tensor.matmul(out=out_ps[:], lhsT=W2_sb[:],
                                     rhs=hidsb[:, :j1 - j0],
                                     start=True, stop=True)
                    nc.scalar.activation(out=osb[:, j0:j1], in_=out_ps[:],
                                         func=AF.Identity,
                                         bias=b2_sb[:, :1], scale=1.0)
                nc.sync.dma_start(out=out_dram[:, :], in_=osb[:])

    return nc


_LAST_EXEC_NS = None


def _run(inputs, trace=False):
    global _LAST_EXEC_NS
    from concourse.bass_utils import run_bass_kernel_spmd

    cfg, shared, per_core = preprocess(**inputs)
    nc = bacc.Bacc("TRN2", target_bir_lowering=False, debug=False,
                   num_devices=M)
    build(nc, cfg)
    nc.compile()

    in_maps = []
    for k in range(M):
        m = dict(shared)
        m.update(per_core[k])
        in_maps.append({k2: np.ascontiguousarray(v) for k2, v in m.items()})

    res = run_bass_kernel_spmd(nc, in_maps, list(range(M)), trace=trace)
    _LAST_EXEC_NS = res.exec_time_ns

    out = np.zeros((N, CLS), np.float32)
    for k in range(M):
        o = res.results[k]["out"]  # [CLS, NPC_PAD]
        out[k * NPC:(k + 1) * NPC] = o[:CLS, :NPC].T
    return out


def kernel(**inputs):
    return _run(inputs, trace=False)


# revision 22
# speedup vs baseline: 1.5954x; 1.0079x over previous
"""DeepGAT (4-layer GAT + BN + residual + MLP head) on 8 Trainium2 cores.

Sharding: nodes are dst-partitioned across the 8 cores (1250 nodes/core).
Edges are routed on the host to the core owning their dst node and sorted by
dst. Weights are replicated. Per layer each core projects all N nodes
(replicated compute), writes per-node rows [xl bf16 x1024 | alpha_src x8 |
pad] to its local DRAM, then gathers the rows of its edges' src nodes with
dma_gather. The per-chunk one-hot (edge, dst) matrices and their transposes
are precomputed on the host and stay SBUF-resident across layers; alpha_dst
is computed per dst block locally and broadcast to edges with a small
ohT @ ad matmul (no per-edge dst gather). Segment softmax normalization is
applied after aggregation (mathematically identical). h is exchanged with a
bf16 AllGather.
"""

import numpy as np
from jax import numpy as jnp

import concourse.bass as bass
import concourse.bacc as bacc
import concourse.mybir as mybir
from concourse.tile import TileContext
from concourse.tile_rust import add_dep_helper

FP32 = mybir.dt.float32
BF16 = mybir.dt.bfloat16
I16 = mybir.dt.int16
AF = mybir.ActivationFunctionType
OP = mybir.AluOpType

# problem constants (hardcoded per harness contract)
ALPHA = 0.1
BN_EPS = 1e-5
NEG_SLOPE = 0.2
HID = 128  # partition width; fixed

N, E, IN, H, L, CLS, M = 10000, 160000, 512, 8, 4, 2, 8
NPC = N // M                      # 1250 nodes per core
NPC_PAD = -(-NPC // 128) * 128    # 1280
NBLK = NPC_PAD // 128             # 10 local dst blocks
N_PAD = -(-N // 128) * 128        # 10112
NNB = N_PAD // 128                # 79 global node blocks
HC = H * HID                      # 1024
PRJ = HC + H                      # 1032: xl | alpha_src
ROW = -(-PRJ // 128) * 128        # 1152 (x2B = 2304B, %256 ok)
SZMAX = 8                         # chunks per gather split (128*8=1024 idxs)

BF = jnp.bfloat16


class Cfg:
    """Static schedule computed from the actual edge data."""

    def __init__(self, chunks_per_block):
        self.chunks_per_block = list(chunks_per_block)
        self.CH = sum(self.chunks_per_block)
        self.TOTE = 128 * self.CH


def _pack_idx16(idx, pad_to=None):
    """Pack int16 indices for dma_gather: idx i at [i%16, i//16], replicated
    to 128 partitions."""
    idx = np.asarray(idx, np.int64)
    n = len(idx)
    if pad_to is not None:
        assert pad_to >= n
        idx = np.concatenate([idx, np.zeros(pad_to - n, np.int64)])
        n = pad_to
    assert n % 16 == 0
    a = idx.astype(np.int16).reshape(n // 16, 16).T  # [16, n//16]
    return np.tile(a, (8, 1)).copy()  # [128, n//16]


def preprocess(x, edge_index, Wp, bp, Wl, att_src, att_dst, bl, gamma, beta,
               W1, b1, W2, b2):
    """Host-side: edge routing/sorting per core + one-hots + weight folding."""
    x = np.asarray(x, np.float32)
    src = np.concatenate([np.asarray(edge_index[0]), np.arange(N)]).astype(np.int64)
    dst = np.concatenate([np.asarray(edge_index[1]), np.arange(N)]).astype(np.int64)

    per_core = []
    for k in range(M):
        m = (dst // NPC) == k
        s_k, d_k = src[m], dst[m] - k * NPC
        order = np.argsort(d_k, kind="stable")
        per_core.append((s_k[order], d_k[order]))

    counts = np.zeros((M, NBLK), np.int64)
    for k in range(M):
        _, d_k = per_core[k]
        b = d_k // 128
        for bb in range(NBLK):
            counts[k, bb] = int((b == bb).sum())
    chunks_per_block = [max(1, int(np.ceil(counts[:, bb].max() / 128)))
                       for bb in range(NBLK)]
    cfg = Cfg(chunks_per_block)
    CH = cfg.CH

    per_core_inputs = []
    for k in range(M):
        s_k, d_k = per_core[k]
        b_k = d_k // 128
        srcidx = np.zeros(cfg.TOTE, np.int64)
        ohs = np.zeros((CH, 128, 128), np.float32)   # [chunk, edge, dstslot]
        off = 0  # in chunks
        for bb in range(NBLK):
            sel = b_k == bb
            cnt = int(sel.sum())
            cap = 128 * cfg.chunks_per_block[bb]
            assert cnt <= cap, (k, bb, cnt, cap)
            srcidx[off * 128:off * 128 + cnt] = s_k[sel]
            dloc = (d_k[sel] - 128 * bb).astype(np.int64)   # 0..127
            mm = np.arange(cnt)
            ohs[off + mm // 128, mm % 128, dloc] = 1.0
            off += cfg.chunks_per_block[bb]
        assert off == CH

        # SBUF layouts: oh [128 edge-part, CH, 128 dst], ohT [128 dst-part, CH, 128 edge]
        oh_sb = np.ascontiguousarray(ohs.transpose(1, 0, 2)).reshape(128, CH * 128)
        ohT_sb = np.ascontiguousarray(ohs.transpose(2, 0, 1)).reshape(128, CH * 128)

        xT_own = np.zeros((IN, NPC_PAD), np.float32)
        xT_own[:, :NPC] = x[k * NPC:(k + 1) * NPC].T

        per_core_inputs.append({
            "srcidx": _pack_idx16(srcidx),
            "oh_in": jnp.asarray(oh_sb, BF),
            "ohT_in": jnp.asarray(ohT_sb, BF),
            "xT_own": jnp.asarray(xT_own, BF),
        })

    # weight folding
    Wl = np.asarray(Wl, np.float32)          # [L, HID, HC]
    a_s = np.asarray(att_src, np.float32)    # [L, H, HID]
    a_d = np.asarray(att_dst, np.float32)
    Wcat = np.zeros((L, HID, PRJ), np.float32)
    Wad = np.zeros((L, HID, H), np.float32)
    for i in range(L):
        # channel-major interleave: col c*H+h holds head h, channel c, so the
        # per-edge attention broadcast has a packed (stride-1) last dim on DVE
        Wcat[i, :, :HC] = Wl[i].reshape(HID, H, HID).transpose(0, 2, 1).reshape(HID, HC)
        w3 = Wl[i].reshape(HID, H, HID)
        Wcat[i, :, HC:] = np.einsum("khc,hc->kh", w3, a_s[i])
        Wad[i] = np.einsum("khc,hc->kh", w3, a_d[i])

    bn_inv = 1.0 / np.sqrt(1.0 + BN_EPS)
    gamma = np.asarray(gamma, np.float32)
    beta = np.asarray(beta, np.float32)
    bl = np.asarray(bl, np.float32)
    # h = elu((1-a)*(gamma*bn_inv*(mean+bl)+beta) + a*prev); fold 1/H into s.
    s_aff = ((1.0 - ALPHA) * gamma * bn_inv / H).T.copy()            # [HID, L]
    t_aff = ((1.0 - ALPHA) * (gamma * bn_inv * bl + beta)).T.copy()  # [HID, L]

    ident = np.eye(128, dtype=np.float32)

    shared = {
        "Wp": jnp.asarray(np.asarray(Wp, np.float32), BF),
        "bp": np.asarray(bp, np.float32)[:, None],
        "Wcat": jnp.asarray(Wcat, BF),
        "Wad": jnp.asarray(Wad, BF),
        "s_aff": s_aff, "t_aff": t_aff,
        "W1": jnp.asarray(np.asarray(W1, np.float32), BF),
        "b1": np.asarray(b1, np.float32)[:, None],
        "W2": jnp.asarray(np.asarray(W2, np.float32), BF),
        "b2": np.asarray(b2, np.float32)[:, None],
        "ident": ident,
    }
    return cfg, shared, per_core_inputs


def _elu(nc, p, out_ap, z_ap, shape, tg):
    """out = elu(z) = relu(z) + exp(min(z,0)) - 1, z in SBUF f32."""
    P, F = shape
    mn = p.tile([P, F], FP32, tag=f"elu_mn_{tg}", name=f"elu_mn_{tg}")
    ex = p.tile([P, F], FP32, tag=f"elu_ex_{tg}", name=f"elu_ex_{tg}")
    rl = p.tile([P, F], FP32, tag=f"elu_rl_{tg}", name=f"elu_rl_{tg}")
    nc.vector.tensor_scalar_min(out=mn[:], in0=z_ap, scalar1=0.0)
    nc.scalar.activation(out=ex[:], in_=mn[:], func=AF.Exp)
    nc.vector.tensor_scalar_max(out=rl[:], in0=z_ap, scalar1=0.0)
    nc.vector.tensor_tensor(out=rl[:], in0=rl[:], in1=ex[:], op=OP.add)
    nc.vector.tensor_scalar_sub(out=out_ap, in0=rl[:], scalar1=1.0)


def build(nc, cfg):
    """Emit the SPMD program."""
    CH = cfg.CH
    qd = HID // 2

    # ---------------- I/O ----------------
    srcidx = nc.dram_tensor("srcidx", [128, cfg.TOTE // 16], I16, kind="ExternalInput")
    oh_in = nc.dram_tensor("oh_in", [128, CH * 128], BF16, kind="ExternalInput")
    ohT_in = nc.dram_tensor("ohT_in", [128, CH * 128], BF16, kind="ExternalInput")
    xT_own_in = nc.dram_tensor("xT_own", [IN, NPC_PAD], BF16, kind="ExternalInput")
    Wp_in = nc.dram_tensor("Wp", [IN, HID], BF16, kind="ExternalInput")
    bp_in = nc.dram_tensor("bp", [HID, 1], FP32, kind="ExternalInput")
    Wcat_in = nc.dram_tensor("Wcat", [L, HID, PRJ], BF16, kind="ExternalInput")
    Wad_in = nc.dram_tensor("Wad", [L, HID, H], BF16, kind="ExternalInput")
    s_aff_in = nc.dram_tensor("s_aff", [HID, L], FP32, kind="ExternalInput")
    t_aff_in = nc.dram_tensor("t_aff", [HID, L], FP32, kind="ExternalInput")
    W1_in = nc.dram_tensor("W1", [HID, qd], BF16, kind="ExternalInput")
    b1_in = nc.dram_tensor("b1", [qd, 1], FP32, kind="ExternalInput")
    W2_in = nc.dram_tensor("W2", [qd, CLS], BF16, kind="ExternalInput")
    b2_in = nc.dram_tensor("b2", [CLS, 1], FP32, kind="ExternalInput")
    ident_in = nc.dram_tensor("ident", [128, 128], FP32, kind="ExternalInput")
    out_dram = nc.dram_tensor("out", [CLS, NPC_PAD], FP32, kind="ExternalOutput")

    xlrow = nc.dram_tensor("xlrow", [N_PAD, ROW], BF16)
    bounce = nc.dram_tensor("h_bounce", [HID, NPC_PAD], BF16)
    agout = nc.dram_tensor("h_agout", [M * HID, NPC_PAD], BF16,
                           addr_space="Shared")

    with TileContext(nc) as tc:
        with (
            tc.tile_pool(name="const", bufs=1) as cpool,
            tc.tile_pool(name="hbuf", bufs=1) as hpool,
            tc.tile_pool(name="proj", bufs=2) as ppool,
            tc.tile_pool(name="gath", bufs=4) as gpool,
            tc.tile_pool(name="ohp", bufs=2) as ohpool,
            tc.tile_pool(name="edge", bufs=2) as epool,
            tc.tile_pool(name="msg", bufs=3) as mpool,
            tc.tile_pool(name="blk", bufs=2) as bpool,
            tc.tile_pool(name="psP", bufs=2, space="PSUM") as psP,
            tc.tile_pool(name="psA", bufs=2, space="PSUM") as psA,
            tc.tile_pool(name="psD", bufs=2, space="PSUM") as psD,
        ):
            _regs = {}

            def nreg(v):
                if v not in _regs:
                    _regs[v] = nc.gpsimd.to_reg(v)
                return _regs[v]

            # ---------------- resident constants / state ----------------
            ident_f = cpool.tile([128, 128], FP32)
            nc.sync.dma_start(out=ident_f[:], in_=ident_in[:, :])
            srcidx_sb = cpool.tile([128, cfg.TOTE // 16], I16)
            nc.sync.dma_start(out=srcidx_sb[:], in_=srcidx[:, :])
            s_aff = cpool.tile([128, L], FP32)
            nc.sync.dma_start(out=s_aff[:], in_=s_aff_in[:, :])
            t_aff = cpool.tile([128, L], FP32)
            nc.sync.dma_start(out=t_aff[:], in_=t_aff_in[:, :])
            W1_sb = cpool.tile([128, qd], BF16)
            nc.sync.dma_start(out=W1_sb[:], in_=W1_in[:, :])
            b1_sb = cpool.tile([qd, 1], FP32)
            nc.sync.dma_start(out=b1_sb[:], in_=b1_in[:, :])
            W2_sb = cpool.tile([qd, CLS], BF16)
            nc.sync.dma_start(out=W2_sb[:], in_=W2_in[:, :])
            b2_sb = cpool.tile([CLS, 1], FP32)
            nc.sync.dma_start(out=b2_sb[:], in_=b2_in[:, :])
            bp_sb = cpool.tile([HID, 1], FP32)
            nc.sync.dma_start(out=bp_sb[:], in_=bp_in[:, :])
            Wc_all = cpool.tile([128, L, PRJ], BF16)
            for li in range(L):
                nc.sync.dma_start(out=Wc_all[:, li, :], in_=Wcat_in[li, :, :])
            Wad_all = cpool.tile([128, L, H], BF16)
            for li in range(L):
                nc.sync.dma_start(out=Wad_all[:, li, :], in_=Wad_in[li, :, :])

            hT = hpool.tile([128, N_PAD], BF16, tag="hT")
            if N_PAD > N:
                nc.vector.memset(hT[:, N:], 0.0)
            h_bf = [hpool.tile([128, NPC_PAD], BF16, tag=f"h_bf{i}",
                               name=f"h_bf{i}")
                    for i in range(2)]

            kchunks = IN // 128

            # ------- h0 = elu(x @ Wp + bp), own nodes only (scoped pool) ----
            with tc.tile_pool(name="x0", bufs=2) as x0pool:
                Wp_sb = x0pool.tile([128, kchunks, HID], BF16, bufs=1)
                for kc in range(kchunks):
                    nc.sync.dma_start(out=Wp_sb[:, kc, :],
                                      in_=Wp_in[kc * 128:(kc + 1) * 128, :])
                for j0 in range(0, NPC_PAD, 512):
                    j1 = min(j0 + 512, NPC_PAD)
                    ps = psP.tile([128, j1 - j0], FP32, tag="p512", name="h0ps")
                    for kc in range(kchunks):
                        xt = x0pool.tile([128, 512], BF16, tag="xT", name="xT")
                        nc.sync.dma_start(
                            out=xt[:, :j1 - j0],
                            in_=xT_own_in[kc * 128:(kc + 1) * 128, j0:j1])
                        nc.tensor.matmul(out=ps[:], lhsT=Wp_sb[:, kc, :],
                                         rhs=xt[:, :j1 - j0],
                                         start=(kc == 0),
                                         stop=(kc == kchunks - 1))
                    z0 = x0pool.tile([128, 512], FP32, tag="z0", name="z0")
                    nc.scalar.activation(out=z0[:, :j1 - j0], in_=ps[:],
                                         func=AF.Identity,
                                         bias=bp_sb[:, :1], scale=1.0)
                    _elu(nc, x0pool, h_bf[0][:, j0:j1], z0[:, :j1 - j0],
                         (128, j1 - j0), "w")

            bw0 = nc.sync.dma_start(out=bounce[:, :], in_=h_bf[0][:])
            prev_bounce_writes = [bw0]
            prev_gathers = []
            prev_readbacks = []

            # ---------------- layers ----------------
            for li in range(L):
                hprev = h_bf[li % 2]
                hnew = h_bf[(li + 1) % 2]

                # --- alpha_dst for own nodes (uses hprev only; overlaps cc) ---
                adall = ppool.tile([128, NBLK, H], BF16, tag="adall")
                for bb in range(NBLK):
                    ps_ad = psD.tile([128, H], FP32, tag="small", name="ps_ad")
                    nc.tensor.matmul(out=ps_ad[:],
                                     lhsT=hprev[:, bb * 128:(bb + 1) * 128],
                                     rhs=Wad_all[:, li, :], start=True, stop=True)
                    nc.scalar.activation(out=adall[:, bb, :], in_=ps_ad[:],
                                         func=AF.Copy)

                # --- allgather h (own cols -> full hT) ---
                cc = nc.gpsimd.collective_compute(
                    "AllGather", OP.bypass,
                    replica_groups=[list(range(M))],
                    ins=[bounce[:, :]], outs=[agout[:, :]],
                )
                # order collective after everything that read/wrote the
                # exchanged buffers last layer (WAR/race fix)
                for w_ in prev_bounce_writes:
                    add_dep_helper(cc.ins, w_.ins, True, "bounce->cc")
                for g_ in prev_gathers:
                    add_dep_helper(cc.ins, g_.ins, True, "gather->cc")
                for r_ in prev_readbacks:
                    add_dep_helper(cc.ins, r_.ins, True, "readback->cc")
                readbacks = []
                for k in range(M):
                    d = nc.sync.dma_start(
                        out=hT[:, k * NPC:(k + 1) * NPC],
                        in_=agout[k * HID:(k + 1) * HID, :NPC])
                    add_dep_helper(d.ins, cc.ins, True, "cc->readback")
                    readbacks.append(d)
                prev_readbacks = readbacks

                # --- projection: all nodes, row = [xl | alpha_src | pad] ---
                Wc = Wc_all[:, li, :]
                tbl_writes = []
                for nb in range(NNB):
                    xlwr = ppool.tile([128, ROW], BF16, tag="xlwr", bufs=4)
                    ps1 = psP.tile([128, 512], FP32, tag="p512", name="ps1")
                    nc.tensor.matmul(out=ps1[:],
                                     lhsT=hT[:, nb * 128:(nb + 1) * 128],
                                     rhs=Wc[:, 0:512], start=True, stop=True)
                    nc.vector.tensor_copy(out=xlwr[:, 0:512], in_=ps1[:])
                    ps2 = psP.tile([128, 512], FP32, tag="p512", name="ps2")
                    nc.tensor.matmul(out=ps2[:],
                                     lhsT=hT[:, nb * 128:(nb + 1) * 128],
                                     rhs=Wc[:, 512:1024], start=True, stop=True)
                    nc.scalar.activation(out=xlwr[:, 512:1024], in_=ps2[:],
                                         func=AF.Copy)
                    ps3 = psD.tile([128, H], FP32, tag="small", name="ps3")
                    nc.tensor.matmul(out=ps3[:],
                                     lhsT=hT[:, nb * 128:(nb + 1) * 128],
                                     rhs=Wc[:, 1024:1032], start=True, stop=True)
                    nc.scalar.activation(out=xlwr[:, 1024:1032], in_=ps3[:],
                                         func=AF.Copy)
                    # cols 1032:1152 are never read downstream; left as-is
                    w_ = nc.sync.dma_start(
                        out=xlrow[nb * 128:(nb + 1) * 128, :], in_=xlwr[:])
                    tbl_writes.append(w_)

                # --- edge phase, per dst block ---
                gathers = []
                bounce_writes = []
                off = 0
                cbmax = max(cfg.chunks_per_block)
                for bb in range(NBLK):
                    cb = cfg.chunks_per_block[bb]
                    splits = []
                    lo = 0
                    while lo < cb:
                        sz = min(SZMAX, cb - lo)
                        splits.append((lo, sz))
                        lo += sz

                    oh_t = ohpool.tile([128, cbmax, 128], BF16, tag="oh",
                                       name="oh_t")
                    nc.sync.dma_start(
                        out=oh_t[:, :cb, :].rearrange("p a b -> p (a b)"),
                        in_=oh_in[:, off * 128:(off + cb) * 128])
                    ohT_t = ohpool.tile([128, cbmax, 128], BF16, tag="ohT",
                                        name="ohT_t")
                    nc.sync.dma_start(
                        out=ohT_t[:, :cb, :].rearrange("p a b -> p (a b)"),
                        in_=ohT_in[:, off * 128:(off + cb) * 128])

                    # alpha_dst broadcast to edges: svd[e, c*8+h]
                    svd_ps = psD.tile([128, cb * H], FP32, tag="small",
                                      name="svd_ps")
                    for c in range(cb):
                        nc.tensor.matmul(out=svd_ps[:, c * H:(c + 1) * H],
                                         lhsT=ohT_t[:, c, :],
                                         rhs=adall[:, bb, :],
                                         start=True, stop=True)

                    den = psD.tile([128, H], FP32, tag="small", name="den")
                    agg = psA.tile([128, HC], FP32, tag="agg")

                    for (lo, sz) in splits:
                        g = gpool.tile([128, SZMAX, ROW], BF16, tag="gt",
                                       name="gt")
                        g1_ = nc.gpsimd.dma_gather(
                            out_ap=g[:, :sz, :], in_ap=xlrow[:, :],
                            idxs_ap=srcidx_sb[:, (off + lo) * 8:(off + lo + sz) * 8],
                            num_idxs=128 * sz, num_idxs_reg=nreg(128 * sz),
                            elem_size=ROW, single_packet=128 * sz <= 1024)
                        for w_ in tbl_writes:
                            add_dep_helper(g1_.ins, w_.ins, True, "tbl->gather")
                        gathers.append(g1_)

                        # u = alpha_src[src] + alpha_dst[dst]  (batched)
                        u = epool.tile([128, SZMAX, H], FP32, tag="u", name="u")
                        nc.vector.tensor_tensor(
                            out=u[:, :sz, :], in0=g[:, :sz, HC:HC + H],
                            in1=svd_ps[:, lo * H:(lo + sz) * H].rearrange(
                                "p (a b) -> p a b", a=sz),
                            op=OP.add)
                        # exp(lrelu(u)) = max(exp(u), exp(slope*u)) since exp
                        # is monotonic; both exps run on the idle scalar engine
                        pe = epool.tile([128, SZMAX, 1, H], BF16, tag="pe",
                                        name="pe")
                        nc.scalar.activation(
                            out=pe[:, :sz].rearrange("p a b c -> p (a b c)"),
                            in_=u[:, :sz].rearrange("p a b -> p (a b)"),
                            func=AF.Exp)
                        pe2 = epool.tile([128, SZMAX, 1, H], BF16, tag="pe2",
                                         name="pe2")
                        nc.scalar.activation(
                            out=pe2[:, :sz].rearrange("p a b c -> p (a b c)"),
                            in_=u[:, :sz].rearrange("p a b -> p (a b)"),
                            func=AF.Exp, scale=NEG_SLOPE)
                        nc.vector.tensor_tensor(
                            out=pe[:, :sz].rearrange("p a b c -> p (a b c)"),
                            in0=pe[:, :sz].rearrange("p a b c -> p (a b c)"),
                            in1=pe2[:, :sz].rearrange("p a b c -> p (a b c)"),
                            op=OP.max)

                        for j in range(sz):
                            c = lo + j
                            # channel-major layout: packed stride-1 head dim
                            # on every operand -> DVE 2x_1p mode
                            msg = mpool.tile([128, HC], BF16, tag="msg",
                                             name="msg")
                            nc.vector.tensor_tensor(
                                out=msg[:].rearrange("p (a b) -> p a b", a=HID),
                                in0=g[:, j, :HC].rearrange(
                                    "p (a b) -> p a b", a=HID),
                                in1=pe[:, j].to_broadcast([128, HID, H]),
                                op=OP.mult)
                            first, last = c == 0, c == cb - 1
                            nc.tensor.matmul(out=den[:],
                                             lhsT=oh_t[:, c, :],
                                             rhs=pe[:, j, 0, :],
                                             start=first, stop=last,
                                             skip_group_check=True)
                            for j0 in range(0, HC, 512):
                                nc.tensor.matmul(out=agg[:, j0:j0 + 512],
                                                 lhsT=oh_t[:, c, :],
                                                 rhs=msg[:, j0:j0 + 512],
                                                 start=first, stop=last,
                                                 skip_group_check=True)
                    off += cb

                    # --- block epilogue (channel-major agg: [d, c*H+h]) ---
                    rec = bpool.tile([128, 1, H], FP32, tag="rec")
                    # pad dst lanes have denom 0; clamp (their agg is 0 too)
                    nc.vector.tensor_scalar_max(out=rec[:, 0, :], in0=den[:],
                                                scalar1=1e-20)
                    nc.vector.reciprocal(out=rec[:, 0, :], in_=rec[:, 0, :])
                    wsum = bpool.tile([128, HID, H], FP32, tag="wsum")
                    nc.vector.tensor_tensor(
                        out=wsum[:],
                        in0=agg[:].rearrange("p (a b) -> p a b", a=HID),
                        in1=rec[:].to_broadcast([128, HID, H]), op=OP.mult)
                    nc.vector.tensor_tensor(out=wsum[:, :, 0:4],
                                            in0=wsum[:, :, 0:4],
                                            in1=wsum[:, :, 4:8], op=OP.add)
                    nc.vector.tensor_tensor(out=wsum[:, :, 0:2],
                                            in0=wsum[:, :, 0:2],
                                            in1=wsum[:, :, 2:4], op=OP.add)
                    hm = bpool.tile([128, 128], FP32, tag="hm")
                    nc.vector.tensor_tensor(out=hm[:], in0=wsum[:, :, 0],
                                            in1=wsum[:, :, 1], op=OP.add)
                    mT_ps = psD.tile([128, 128], FP32, tag="small",
                                     name="mT_ps")
                    nc.tensor.transpose(out=mT_ps[:], in_=hm[:],
                                        identity=ident_f[:])
                    z1 = bpool.tile([128, 128], FP32, tag="z1")
                    nc.scalar.activation(out=z1[:], in_=mT_ps[:],
                                         func=AF.Identity,
                                         bias=t_aff[:, li:li + 1],
                                         scale=s_aff[:, li:li + 1])
                    z2 = bpool.tile([128, 128], FP32, tag="z2")
                    nc.vector.tensor_scalar_mul(
                        out=z2[:], in0=hprev[:, bb * 128:(bb + 1) * 128],
                        scalar1=ALPHA)
                    nc.vector.tensor_tensor(out=z1[:], in0=z1[:], in1=z2[:],
                                            op=OP.add)
                    _elu(nc, bpool, hnew[:, bb * 128:(bb + 1) * 128], z1[:],
                         (128, 128), "n")
                    if li < L - 1:
                        bw = nc.sync.dma_start(
                            out=bounce[:, bb * 128:(bb + 1) * 128],
                            in_=hnew[:, bb * 128:(bb + 1) * 128])
                        add_dep_helper(bw.ins, cc.ins, True, "cc->bounce")
                        bounce_writes.append(bw)
                prev_gathers = gathers
                prev_bounce_writes = bounce_writes

            # ---------------- classifier ----------------
            hfin = h_bf[L % 2]
            with tc.tile_pool(name="cls", bufs=2) as wpool:
                osb = wpool.tile([CLS, NPC_PAD], FP32, tag="osb", bufs=1)
                for j0 in range(0, NPC_PAD, 512):
                    j1 = min(j0 + 512, NPC_PAD)
                    hid_ps = psP.tile([qd, j1 - j0], FP32, tag="p512",
                                      name="hid_ps")
                    nc.tensor.matmul(out=hid_ps[:], lhsT=W1_sb[:],
                                     rhs=hfin[:, j0:j1], start=True, stop=True)
                    zc = wpool.tile([qd, 512], FP32, tag="zc", name="zc")
                    nc.scalar.activation(out=zc[:, :j1 - j0], in_=hid_ps[:],
                                         func=AF.Identity,
                                         bias=b1_sb[:, :1], scale=1.0)
                    hidsb = wpool.tile([qd, 512], BF16, tag="hidsb",
                                       name="hidsb")
                    _elu(nc, wpool, hidsb[:, :j1 - j0], zc[:, :j1 - j0],
                         (qd, j1 - j0), "c")
                    out_ps = psP.tile([CLS, j1 - j0], FP32, tag="p512",
                                      name="out_ps")
                    nc.tensor.matmul(out=out_ps[:], lhsT=W2_sb[:],
                                     rhs=hidsb[:, :j1 - j0],
                                     start=True, stop=True)
                    nc.scalar.activation(out=osb[:, j0:j1], in_=out_ps[:],
                                         func=AF.Identity,
                                         bias=b2_sb[:, :1], scale=1.0)
                nc.sync.dma_start(out=out_dram[:, :], in_=osb[:])

    return nc


_LAST_EXEC_NS = None


def _run(inputs, trace=False):
    global _LAST_EXEC_NS
    from concourse.bass_utils import run_bass_kernel_spmd

    cfg, shared, per_core = preprocess(**inputs)
    nc = bacc.Bacc("TRN2", target_bir_lowering=False, debug=False,
                   num_devices=M)
    build(nc, cfg)
    nc.compile()

    in_maps = []
    for k in range(M):
        m = dict(shared)
        m.update(per_core[k])
        in_maps.append({k2: np.ascontiguousarray(v) for k2, v in m.items()})

    res = run_bass_kernel_spmd(nc, in_maps, list(range(M)), trace=trace)
    _LAST_EXEC_NS = res.exec_time_ns

    out = np.zeros((N, CLS), np.float32)
    for k in range(M):
        o = res.results[k]["out"]  # [CLS, NPC_PAD]
        out[k * NPC:(k + 1) * NPC] = o[:CLS, :NPC].T
    return out


def kernel(**inputs):
    return _run(inputs, trace=False)
